# revision 47
# baseline (speedup 1.0000x reference)
"""Self-contained Trainium2 Bass kernel for nn_EncoderDecoderTransformer_90941637525663.

Sequence-parallel over 8 NeuronCores (2 batch groups x 4 token shards of 256
tokens). Activations live TRANSPOSED in SBUF (feature dim on partitions,
tokens on free dim). One AllGather per layer exchanges self-attention K/V
shards (raw V + rinv row; receivers apply sender rinv * visibility mask).
All heavy matmuls run in bf16 (weights cast host-side). Normalization chains
are batched: one reciprocal per q/k pass and per attention softmax instead of
per-head chains. RoPE's half-rotation uses an on-chip permutation matmul
instead of SBUF-SBUF DMA swaps. LN scales are folded into consumer weights
host-side; deferred-LN per-head rms folds rinv in via rv2.
"""
import sys
sys.path.insert(0, '/opt/trn_rl_repo')
import numpy as np

B, TQ, TK, D, H, KVH, L, F = 2, 1024, 512, 1024, 16, 4, 2, 4096
HD, KVD = 64, 256
EPS = 1e-6
NCORES, TP = 8, 4
T = TQ // TP           # 256 tokens per core (interleaved: t = 4j + r)
DCH = D // 128         # 8 feature chunks
NEG = -3.0e38
AGR = 2 * T            # ag rows: [0,T)=k (normalized), [T,2T)=v (rinv-scaled)


def _bf16():
    import ml_dtypes
    return ml_dtypes.bfloat16


def _rope_tables(Tlen, hd, theta=10000.0):
    freqs = 1.0 / theta ** (np.arange(0, hd, 2, dtype=np.float32) / hd)
    ang = np.outer(np.arange(Tlen, dtype=np.float32), freqs)
    return np.cos(ang).astype(np.float32), np.sin(ang).astype(np.float32)


def host_prepare(inputs):
    """Returns (host, per_core): folded shared arrays + per-core arrays."""
    bf16 = _bf16()
    inputs = {k: np.ascontiguousarray(np.asarray(v, dtype=np.float32))
              for k, v in inputs.items()}
    cos_f, sin_f = _rope_tables(TQ, HD)       # [TQ, 32]

    def bf(a):
        return np.ascontiguousarray(a.astype(bf16))

    fp16 = np.float16

    host = {}
    for i in range(L):
        ln1 = (1.0 + inputs['ln1_s'][i])[:, None]
        ln2 = (1.0 + inputs['ln2_s'][i])[:, None]
        ln3 = (1.0 + inputs['ln3_s'][i])[:, None]
        host[f'sa_wq_{i}'] = bf(ln1 * inputs['sa_wq'][i])
        host[f'sa_wk_{i}'] = bf(ln1 * inputs['sa_wk'][i])
        host[f'sa_wv_{i}'] = bf(ln1 * inputs['sa_wv'][i])
        host[f'sa_wo_{i}'] = bf(inputs['sa_wo'][i])
        host[f'ca_wq_{i}'] = bf(ln2 * inputs['ca_wq'][i])
        host[f'ca_wk_{i}'] = bf(inputs['ca_wk'][i])
        host[f'ca_wv_{i}'] = bf(inputs['ca_wv'][i])
        host[f'ca_wo_{i}'] = bf(inputs['ca_wo'][i])
        host[f'ffn_wg_{i}'] = bf(ln3 * inputs['ffn_wg'][i])
        host[f'ffn_wu_{i}'] = bf(ln3 * inputs['ffn_wu'][i])
        host[f'ffn_wd_{i}'] = bf(inputs['ffn_wd'][i])
        # rope tables with (1+qn)/(1+kn) head-dim scales folded; [128, TQ]
        for which, dvec in [('q', inputs['sa_qn'][i]), ('k', inputs['sa_kn'][i])]:
            d1, d2 = 1.0 + dvec[:32], 1.0 + dvec[32:]
            C = np.concatenate([d1[:, None] * cos_f.T, d2[:, None] * cos_f.T], 0)
            S = np.concatenate([-d2[:, None] * sin_f.T, d1[:, None] * sin_f.T], 0)
            host[f'rope{which}_c_{i}'] = np.ascontiguousarray(np.concatenate([C, C], 0))
            host[f'rope{which}_s_{i}'] = np.ascontiguousarray(np.concatenate([S, S], 0))
        sc = ((1.0 + inputs['ca_qn'][i]) * (1.0 + inputs['ca_kn'][i])).astype(np.float32)
        host[f'ca_kscale_{i}'] = np.tile(sc, KVH)[:, None].copy()   # [256, 1]
    host['final_scale'] = (1.0 + inputs['final_s'])[:, None].copy()  # [D, 1]
    # bdall[:, mt, :]: [128,128] stationary (fp16, M padded to 128 for fast
    # weight load) accumulating chunk mt's per-head-pair row sums into rows
    # 2mt/2mt+1 of a [128,T] psum (other rows zero)
    bdall = np.zeros((128, 8, 128), fp16)
    for mt in range(8):
        bdall[0:64, mt, 2 * mt] = 1.0
        bdall[64:128, mt, 2 * mt + 1] = 1.0
    host['bdall'] = bdall
    # sel16[0:n, mt, :]: [n,128] stationary (fp16) broadcasting rows 2mt/2mt+1
    # of an [n,T] scale tile to the 64-row head halves of a [128,T] output
    sel16 = np.zeros((16, 8, 128), fp16)
    for mt in range(8):
        sel16[2 * mt, mt, 0:64] = 1.0
        sel16[2 * mt + 1, mt, 64:128] = 1.0
    host['sel16'] = sel16
    # rope half-swap permutation (within each 64-row head block): [128, 128]
    psw = np.zeros((128, 128), np.float32)
    for base in (0, 64):
        for r in range(32):
            psw[base + 32 + r, base + r] = 1.0      # out row base+r <- in row base+32+r
            psw[base + r, base + 32 + r] = 1.0      # out row base+32+r <- in row base+r
    host['pswap'] = bf(psw)

    per_core = []
    kl = np.arange(128)[:, None]
    ql = np.arange(128)[None, :]
    tri_inc = np.where(kl <= ql, 0.0, NEG).astype(np.float32)
    tri_exc = np.where(kl < ql, 0.0, NEG).astype(np.float32)
    for c in range(NCORES):
        grp, r = c // TP, c % TP
        tok = slice(r, None, TP)   # interleaved tokens: 4j + r
        pc = {
            'xT': np.ascontiguousarray(inputs['x'][grp].T[:, tok]),
            'encTb': bf(inputs['encoder_out'][grp].T),
        }
        # diagonal-chunk additive mask vs source core g: k-token 4p+g visible
        # to q-token 4j+r within the same 128-block iff p<=j (g<=r) / p<j (g>r)
        dm = np.empty((128, TP, 128), np.float32)
        for g in range(TP):
            dm[:, g, :] = tri_inc if g <= r else tri_exc
        pc['dmask'] = np.ascontiguousarray(dm)
        for i in range(L):
            for nm in ('ropeq_c', 'ropeq_s', 'ropek_c', 'ropek_s'):
                pc[f'{nm}_{i}'] = np.ascontiguousarray(host[f'{nm}_{i}'][:, tok])
        per_core.append(pc)
    return host, per_core


_PROG = None
DBG = False


def _build_program(nlayers=L):
    import concourse.bass as bass
    import concourse.tile as tile
    from concourse import bacc, mybir
    from concourse.alu_op_type import AluOpType
    from contextlib import ExitStack

    R = mybir.dt.float32r
    FP = mybir.dt.float32
    BF = mybir.dt.bfloat16
    F16 = mybir.dt.float16
    Exp = mybir.ActivationFunctionType.Exp
    Ln = mybir.ActivationFunctionType.Ln
    Relu = mybir.ActivationFunctionType.Relu
    Square = mybir.ActivationFunctionType.Square
    Copy = mybir.ActivationFunctionType.Copy

    class _Bacc(bacc.Bacc):
        def insert_act_table_loads(self):
            # Force exp/ln (this kernel's only table-bound activations) to
            # resolve to the combined natural_log_exp_and_others set: the
            # default first-match selection alternates exp_and_others /
            # natural_log and pays a ~1.3us table load per switch.
            import bass_rust as _bass_rust
            has_activation = any(
                isinstance(ins, mybir.InstActivation)
                for b in self.main_func.blocks
                for ins in b.instructions
            )
            if not has_activation:
                return
            from concourse.hw_specs import get_activation_tables
            tables = list(get_activation_tables(self.m.arch).items())
            combined = next(i for i, (nm, _) in enumerate(tables)
                            if nm == 'natural_log_exp_and_others')
            tables = [(nm, s if i == combined else set())
                      for i, (nm, s) in enumerate(tables)]
            _bass_rust.insert_act_table_loads(self, tables)

    nc = _Bacc("TRN2", target_bir_lowering=False, debug=False,
               num_devices=NCORES)

    din = {}
    def dri(name, shape, dt):
        din[name] = nc.dram_tensor(name, list(shape), dt, kind="ExternalInput")

    dri('xT', (D, T), FP)
    dri('encTb', (D, TK), BF)
    dri('dmask', (128, TP, 128), FP)
    dri('final_scale', (D, 1), FP)
    dri('bdall', (128, 8, 128), F16)
    dri('sel16', (16, 8, 128), F16)
    dri('pswap', (128, 128), BF)
    for i in range(nlayers):
        dri(f'sa_wq_{i}', (D, D), BF); dri(f'sa_wk_{i}', (D, KVD), BF)
        dri(f'sa_wv_{i}', (D, KVD), BF); dri(f'sa_wo_{i}', (D, D), BF)
        dri(f'ca_wq_{i}', (D, D), BF); dri(f'ca_wk_{i}', (D, KVD), BF)
        dri(f'ca_wv_{i}', (D, KVD), BF); dri(f'ca_wo_{i}', (D, D), BF)
        dri(f'ffn_wg_{i}', (D, F), BF); dri(f'ffn_wu_{i}', (D, F), BF)
        dri(f'ffn_wd_{i}', (F, D), BF)
        for nm in ('ropeq_c', 'ropeq_s', 'ropek_c', 'ropek_s'):
            dri(f'{nm}_{i}', (128, T), FP)
        dri(f'ca_kscale_{i}', (KVD, 1), FP)
    out_dram = nc.dram_tensor('outT', [D, T], FP, kind="ExternalOutput")
    dbg = {}
    if DBG:
        for nm, shp, dt in [('tap_rbc', [128, T], FP), ('tap_xb', [128, T], BF),
                            ('tap_qsc', [16, T], FP), ('tap_qf', [128, T], BF),
                            ('tap_kd', [128, T], BF), ('tap_vown', [128, 256], BF),
                            ('tap_kfull', [128, TQ], BF), ('tap_vfull', [128, TP * 2 * 128], BF),
                            ('tap_rvg', [128, TP * 2], BF), ('tap_ao', [128, T], BF),
                            ('tap_x1', [128, T], FP)]:
            dbg[nm] = nc.dram_tensor(nm, shp, dt, kind="ExternalOutput")
    ag_in = [nc.dram_tensor(f'ag_in_{i}', [AGR, T], BF) for i in range(nlayers)]
    ag_out_k = [nc.dram_tensor(f'ag_out_k_{i}', [T * TP, T], BF) for i in range(nlayers)]
    ag_out_v = [nc.dram_tensor(f'ag_out_v_{i}', [T * TP, T], BF) for i in range(nlayers)]
    agw_in = nc.dram_tensor('agw_in', [1, 128], BF)
    agw_out = nc.dram_tensor('agw_out', [TP, 128], BF)
    GROUPS = [[0, 1, 2, 3], [4, 5, 6, 7]]

    with nc.allow_low_precision(reason="bf16 pipeline"), \
            tile.TileContext(nc) as tc, ExitStack() as ctx:
        consts = ctx.enter_context(tc.tile_pool(name="consts", bufs=1))
        state = ctx.enter_context(tc.tile_pool(name="state", bufs=1))
        aqp = ctx.enter_context(tc.tile_pool(name="aqp", bufs=1))
        kvf = ctx.enter_context(tc.tile_pool(name="kvf", bufs=2))
        wbig = ctx.enter_context(tc.tile_pool(name="wbig", bufs=3))
        wkv = ctx.enter_context(tc.tile_pool(name="wkv", bufs=2))
        workA = ctx.enter_context(tc.tile_pool(name="workA", bufs=2))
        workB = ctx.enter_context(tc.tile_pool(name="workB", bufs=2))
        psb = ctx.enter_context(tc.tile_pool(name="psb", bufs=4))
        ffnp = ctx.enter_context(tc.tile_pool(name="ffnp", bufs=2))
        ropep = ctx.enter_context(tc.tile_pool(name="ropep", bufs=2))
        ps = ctx.enter_context(tc.tile_pool(name="ps", bufs=8, space="PSUM"))

        def pst(p_, f_, name, tag="s"):
            bufs = {"s": 3, "o": 3, "m": 2}[tag]
            return ps.tile([p_, f_], FP, tag=f"ps_{tag}", name=name, bufs=bufs)

        MUL, ADD = AluOpType.mult, AluOpType.add

        # ---- warm-up collective: the first collective's entry barrier
        # absorbs inter-core launch skew; run it on a tiny buffer up front so
        # the wait overlaps constant/weight loads and the pre-AG compute ----
        warm = consts.tile([1, 128], BF, tag="warm", name="warm")
        nc.vector.memset(warm[:], 0.0)
        nc.sync.dma_start(out=agw_in.ap(), in_=warm[:])
        nc.gpsimd.collective_compute(
            "AllGather", mybir.AluOpType.bypass, replica_groups=GROUPS,
            ins=[agw_in.ap().opt()], outs=[agw_out.ap().opt()])

        # ---- constants ----
        ones = consts.tile([128, 128], F16, tag="ones", name="ones")
        nc.vector.memset(ones[:], 1.0)
        bdall = consts.tile([128, 8, 128], F16, tag="bdall", name="bdall")
        nc.sync.dma_start(out=bdall[:], in_=din['bdall'].ap())
        sel16 = consts.tile([16, 8, 128], F16, tag="sel16", name="sel16")
        nc.sync.dma_start(out=sel16[:], in_=din['sel16'].ap())
        eps_t = consts.tile([128, 1], FP, tag="eps", name="eps")
        nc.vector.memset(eps_t[:], EPS)
        id1 = consts.tile([1, 1], FP, tag="id1", name="id1")
        nc.vector.memset(id1[:], 1.0)
        pswap = consts.tile([128, 128], BF, tag="pswap", name="pswap")
        nc.sync.dma_start(out=pswap[:], in_=din['pswap'].ap())
        dmask = consts.tile([128, TP, 128], FP, tag="dmask", name="dmask")
        nc.sync.dma_start(out=dmask[:], in_=din['dmask'].ap())
        fscale = consts.tile([128, DCH], FP, tag="fscale", name="fscale")
        nc.sync.dma_start(out=fscale[:],
                          in_=din['final_scale'].ap().rearrange("(k p) o -> p (k o)", p=128))

        # ---- persistent state ----
        x = [state.tile([128, T], FP, tag=f"x{m}", name=f"x{m}") for m in range(DCH)]
        for m in range(DCH):
            nc.sync.dma_start(out=x[m][:], in_=din['xT'].ap()[128 * m:128 * (m + 1), :])
        xb = [state.tile([128, T], BF, tag=f"xb{m}", name=f"xb{m}") for m in range(DCH)]
        encb = [state.tile([128, TK], BF, tag=f"enc{m}", name=f"enc{m}") for m in range(DCH)]
        for m in range(DCH):
            nc.sync.dma_start(out=encb[m][:], in_=din['encTb'].ap()[128 * m:128 * (m + 1), :])
        ao = [state.tile([128, T], BF, tag=f"ao{m}", name=f"ao{m}") for m in range(DCH)]
        qf = [state.tile([128, T], BF, tag=f"qf{t}", name=f"qf{t}") for t in range(H // 2)]
        kdca = [state.tile([128, TK], BF, tag=f"kdca{k}", name=f"kdca{k}") for k in range(KVH)]
        cv = [state.tile([128, TP, 128], BF, tag=f"cv{k}", name=f"cv{k}") for k in range(KVH)]
        y_sb = [state.tile([128, T], FP, tag=f"ysb{m}", name=f"ysb{m}") for m in range(DCH)]
        aq = y_sb

        # ---------------- helpers ----------------
        def ln_prep(tagp=""):
            """rms over D partitions of fp32 x -> (rbc [128,T] FP broadcast of
            1/rms, rv2 [128,T] FP = rbc^2). Also refreshes xb (bf16 cast of x).
            Uses exp(-0.5*ln(mean+eps)) on full 128 rows so no partition
            broadcast is needed (ACT cost is free-dim bound)."""
            ss = pst(128, T, "ss", tag="o")
            for m in range(DCH):
                sq = workA.tile([128, T], F16, tag="sq", name="sq")
                nc.vector.tensor_tensor(sq[:], x[m][:], x[m][:], MUL)
                nc.tensor.matmul(ss[:], lhsT=ones[:], rhs=sq[:],
                                 start=(m == 0), stop=(m == DCH - 1))
                nc.scalar.activation(xb[m][:], x[m][:], Copy)
            lnv = workA.tile([128, T], FP, tag="lnv" + tagp, name="lnv")
            nc.scalar.activation(lnv[:], ss[:], Ln, bias=eps_t[:], scale=1.0 / D)
            rbc = workA.tile([128, T], FP, tag="rbc" + tagp, name="rbc")
            nc.scalar.activation(rbc[:], lnv[:], Exp, scale=-0.5)
            rv2 = workA.tile([128, T], FP, tag="rv2" + tagp, name="rv2")
            nc.vector.tensor_tensor(rv2[:], rbc[:], rbc[:], MUL)
            return rbc, rv2

        def batch_norm_scale(ssq_ps, nrows, rv2=None, width=T):
            """Per-head-pair raw sums [nrows, width] psum -> 1/rms scale (F16)
            via exp(-0.5*ln(mean+eps)). With rv2: deferred-LN variant (sums
            scaled by rinv^2 first)."""
            if rv2 is not None:
                tk = workA.tile([nrows, width], FP, tag="bnt", name="bnt",
                                padded_shape=[16, TK])
                nc.vector.tensor_tensor(tk[:], ssq_ps[0:nrows, 0:width],
                                        rv2[0:nrows, :width], MUL)
                src = tk[:]
            else:
                src = ssq_ps[0:nrows, 0:width]
            sr = workB.tile([nrows, width], FP, tag="bns", name="bns",
                            padded_shape=[16, TK])
            nc.scalar.activation(sr[:], src, Ln, bias=eps_t[0:nrows, :], scale=1.0 / HD)
            rr = workB.tile([nrows, width], F16, tag="bnr", name="bnr",
                            padded_shape=[16, TK])
            nc.scalar.activation(rr[:], sr[:], Exp, scale=-0.5)
            return rr

        def pair_bcast(scale_tile, t, nrows=16, width=T, rbc=None):
            """Broadcast rows 2t/2t+1 of an [nrows, width] F16 scale tile to the
            64-row halves of a [128, width] FP tile via PE selector matmul.
            If rbc given, fold the per-token rinv into the drain."""
            bc_ps = pst(128, width, "bch", tag="m")
            nc.tensor.matmul(bc_ps[:], lhsT=sel16[0:nrows, t, :],
                             rhs=scale_tile[:],
                             start=True, stop=True)
            rbch = workA.tile([128, width], FP, tag="rbch", name="rbch",
                              padded_shape=[128, TK])
            if rbc is not None:
                nc.vector.tensor_tensor(rbch[:], bc_ps[:], rbc[:, :width], MUL)
            else:
                nc.scalar.activation(rbch[:], bc_ps[:], Copy)
            return rbch

        def norm_pair(qt, o2):
            """Divide both heads' outputs by their softmax rowsums (row 64 of
            each o2): recip on vector, partition-broadcast on the otherwise
            idle gpsimd, final muls on vector."""
            rs2 = workB.tile([1, 2, T], FP, tag="rs2", name="rs2", bufs=3)
            nc.scalar.activation(rs2[0:1, 0, :], o2[0][64:65, :], Copy)
            nc.vector.tensor_copy(rs2[0:1, 1, :], o2[1][64:65, :])
            rr2 = workB.tile([1, 2, T], FP, tag="rr2", name="rr2", bufs=3)
            nc.vector.reciprocal_approx_fast(rr2[:], rs2[:])
            bc_sb = workB.tile([128, 2, T], FP, tag="bc_sb", name="bc_sb", bufs=3)
            nc.gpsimd.partition_broadcast(bc_sb[:], rr2[:])
            for h, par in ((0, 0), (1, 64)):
                nc.vector.tensor_tensor(ao[qt][par:par + 64, :],
                                        o2[h][0:64, :],
                                        bc_sb[par:par + 64, h, :], MUL)

        def sa_attention(i):
            agk = ag_out_k[i].ap()
            agv = ag_out_v[i].ap()
            for kv in range(KVH):
                kga = kvf.tile([128, TP, T], BF, tag="kfull", name="kfull")
                src = bass.AP(tensor=agk.tensor, offset=(64 * kv) * T,
                              ap=[[T, 64], [T * T, TP], [1, T]])
                for dd in range(2):
                    nc.sync.dma_start(out=kga[64 * dd:64 * (dd + 1), :, :], in_=src)
                vga = kvf.tile([128, TP, 2, 128], BF, tag="vfull", name="vfull")
                nc.vector.memset(vga[:, :, :, 64:65], 1.0)
                nc.vector.memset(vga[:, :, :, 65:128], 0.0)
                for cb in range(2):
                    vap = bass.AP(tensor=agv.tensor,
                                  offset=128 * T * cb + 64 * kv,
                                  ap=[[T, 128], [T * T, TP], [1, 64]])
                    nc.sync.dma_start(out=vga[:, :, cb, 0:64], in_=vap)
                for pp in range(2):
                    qt = 2 * kv + pp
                    o2 = [pst(128, T, f"o{h}", tag="o") for h in range(2)]
                    mi = 0
                    # C=0 chunks [128k x 256q]: left half (vs J0) is the
                    # diagonal (masked), right half (vs J1) fully visible
                    for b in range(2):
                        s2t = [pst(128, 512, "s") for _ in range(2)]
                        for gg in range(2):
                            g = 2 * b + gg
                            for h, par in ((0, 0), (1, 64)):
                                nc.tensor.matmul(
                                    s2t[h][:, 256 * gg:256 * (gg + 1)],
                                    lhsT=kga[par:par + 64, g, 0:128],
                                    rhs=qf[qt][par:par + 64, :],
                                    start=True, stop=True)
                        p2t = []
                        for h in range(2):
                            sv = s2t[h][:].rearrange("p (c t) -> p c t", c=2)
                            nc.vector.tensor_tensor(
                                sv[:, :, 0:128], sv[:, :, 0:128],
                                dmask[:, 2 * b:2 * b + 2, :], ADD)
                            p_sb = psb.tile([128, 2, T], BF, tag="p_sb", name="p_sb", bufs=6)
                            nc.scalar.activation(p_sb[:], sv, Exp, scale=0.125)
                            p2t.append(p_sb)
                        for gg in range(2):
                            g = 2 * b + gg
                            for h in range(2):
                                nc.tensor.matmul(o2[h][:], lhsT=vga[:, g, 0, :],
                                                 rhs=p2t[h][:, gg, :],
                                                 start=(mi == 0), stop=False,
                                                 skip_group_check=True)
                            mi += 1
                    # C=1 chunks [128k x 128q]: all diagonal, right half only
                    s2t = [pst(128, 512, "s") for _ in range(2)]
                    for h, par in ((0, 0), (1, 64)):
                        for g in range(TP):
                            nc.tensor.matmul(
                                s2t[h][:, 128 * g:128 * (g + 1)],
                                lhsT=kga[par:par + 64, g, 128:256],
                                rhs=qf[qt][par:par + 64, 128:256],
                                start=True, stop=True)
                    p2t = []
                    for h in range(2):
                        sv = s2t[h][:].rearrange("p (g t) -> p g t", g=TP)
                        nc.vector.tensor_tensor(sv, sv, dmask[:], ADD)
                        p_sb = psb.tile([128, TP, 128], BF, tag="p_sb", name="p_sb", bufs=6)
                        nc.scalar.activation(p_sb[:], sv, Exp, scale=0.125)
                        p2t.append(p_sb)
                    for g in range(TP):
                        for h in range(2):
                            nc.tensor.matmul(o2[h][:, 128:256], lhsT=vga[:, g, 1, :],
                                             rhs=p2t[h][:, g, :],
                                             start=False, stop=(g == TP - 1),
                                             skip_group_check=True)
                    norm_pair(qt, o2)

        def ca_attention(i):
            nk_chunks = TK // 128
            for kv in range(KVH):
                ksrc = kdca[kv]
                vsrc = cv[kv]
                for pp in range(2):
                    # process both heads of pair tile qt together: their score
                    # matmuls use disjoint PE row groups (lhsT base 0 vs 64) and
                    # overlap when issued back-to-back
                    qt = 2 * kv + pp
                    o2 = [pst(128, T, f"o{h}", tag="o") for h in range(2)]
                    mi = 0
                    for half in range(nk_chunks // 2):
                        s2t = [pst(128, 512, "s") for _ in range(2)]
                        for cc in range(2):
                            c = 2 * half + cc
                            for h, par in ((0, 0), (1, 64)):
                                nc.tensor.matmul(
                                    s2t[h][:, 256 * cc:256 * (cc + 1)],
                                    lhsT=ksrc[par:par + 64, 128 * c:128 * (c + 1)],
                                    rhs=qf[qt][par:par + 64, :], start=True, stop=True)
                        p2t = []
                        for h in range(2):
                            p_sb = psb.tile([128, 2, T], BF, tag="p_sb", name="p_sb", bufs=6)
                            nc.scalar.activation(p_sb[:], s2t[h][:].rearrange(
                                "p (c t) -> p c t", c=2), Exp, scale=0.125)
                            p2t.append(p_sb)
                        for cc in range(2):
                            c = 2 * half + cc
                            for h in range(2):
                                nc.tensor.matmul(o2[h][:], lhsT=vsrc[:, c, :],
                                                 rhs=p2t[h][:, cc, :],
                                                 start=(mi == 0),
                                                 stop=(mi == nk_chunks - 1),
                                                 skip_group_check=True)
                            mi += 1
                    norm_pair(qt, o2)

        def stream_out_proj(w_name):
            for bki in range(2):
                wt = wbig.tile([128, DCH, 512], BF, tag="wbig", name="wbig")
                nc.sync.dma_start(
                    out=wt[:],
                    in_=din[w_name].ap().rearrange("(k p) n -> p k n", p=128)
                    [:, :, bki * 512:(bki + 1) * 512])
                for j in range(4):
                    m = bki * 4 + j
                    y_ps = pst(128, T, "proj", tag="m")
                    for k in range(DCH):
                        nc.tensor.matmul(y_ps[:], lhsT=wt[:, k, 128 * j:128 * (j + 1)],
                                         rhs=ao[k][:],
                                         start=(k == 0), stop=(k == DCH - 1))
                    nc.vector.tensor_tensor(x[m][:], x[m][:], y_ps[:], ADD)

        # ================= layers =================
        for i in range(nlayers):
            # ---- LN1 (deferred) + SA ----
            rbc, rv2 = ln_prep()
            rvT = []
            for j in range(2):
                tp_ps = pst(128, 1, "tp", tag="m")
                nc.tensor.transpose(tp_ps[:], rbc[0:1, 128 * j:128 * (j + 1)], id1[:])
                rv_b = workB.tile([128, 1], FP, tag=f"rvT{j}", name=f"rvT{j}")
                nc.scalar.activation(rv_b[:], tp_ps[:], Copy)
                rvT.append(rv_b)

            rtq_c = ropep.tile([128, T], FP, tag="rtc", name="rtc")
            nc.sync.dma_start(out=rtq_c[:], in_=din[f'ropeq_c_{i}'].ap())
            rtq_s = ropep.tile([128, T], FP, tag="rts", name="rts")
            nc.sync.dma_start(out=rtq_s[:], in_=din[f'ropeq_s_{i}'].ap())
            rtk_c = ropep.tile([128, T], FP, tag="rtc", name="rtc")
            nc.sync.dma_start(out=rtk_c[:], in_=din[f'ropek_c_{i}'].ap())
            rtk_s = ropep.tile([128, T], FP, tag="rts", name="rts")
            nc.sync.dma_start(out=rtk_s[:], in_=din[f'ropek_s_{i}'].ap())

            # -- SA K (2 chunks of 128 cols) --
            wkt = wkv.tile([128, DCH, KVD], BF, tag="wkv", name="wkv")
            nc.sync.dma_start(out=wkt[:],
                              in_=din[f'sa_wk_{i}'].ap().rearrange("(k p) n -> p k n", p=128))
            ssqk_ps = pst(128, T, "ssqk", tag="o")
            ka = []
            for t in range(2):
                k_ps = pst(128, T, "kproj")
                for k in range(DCH):
                    nc.tensor.matmul(k_ps[:], lhsT=wkt[:, k, 128 * t:128 * (t + 1)],
                                     rhs=xb[k][:], start=(k == 0), stop=(k == DCH - 1))
                kraw = workA.tile([128, T], BF, tag="kraw", name="kraw")
                nc.scalar.activation(kraw[:], k_ps[:], Copy)
                ksw_ps = pst(128, T, "ksw", tag="m")
                nc.tensor.matmul(ksw_ps[:], lhsT=pswap[:], rhs=kraw[:],
                                 start=True, stop=True)
                sqk = workA.tile([128, T], F16, tag="sqk", name="sqk")
                nc.scalar.activation(sqk[:], k_ps[:], Square)
                nc.tensor.matmul(ssqk_ps[:], lhsT=bdall[:, t, :],
                                 rhs=sqk[:], start=(t == 0), stop=(t == 1))
                kat = workB.tile([128, T], FP, tag=f"ka{t}", name=f"ka{t}")
                nc.vector.tensor_tensor(kat[:], k_ps[:], rtk_c[:], MUL)
                ksw_m = workA.tile([128, T], FP, tag="kswm", name="kswm")
                nc.vector.tensor_tensor(ksw_m[:], ksw_ps[:], rtk_s[:], MUL)
                nc.vector.tensor_tensor(kat[:], kat[:], ksw_m[:], ADD)
                ka.append(kat)
            ksc = batch_norm_scale(ssqk_ps, 4, rv2=rv2)
            for t in range(2):
                rbch = pair_bcast(ksc, t, nrows=4, rbc=rbc)
                kf = workA.tile([128, T], BF, tag="kf", name="kf")
                nc.vector.tensor_tensor(kf[:], ka[t][:], rbch[:], MUL)
                nc.sync.dma_start(out=ag_in[i].ap()[128 * t:128 * (t + 1), :],
                                  in_=kf[:])
            # gather K on its own (scores only need K; V follows in a second
            # collective that overlaps with the first wave of score matmuls)
            nc.gpsimd.collective_compute(
                "AllGather", mybir.AluOpType.bypass, replica_groups=GROUPS,
                ins=[ag_in[i].ap()[0:T, :].opt()], outs=[ag_out_k[i].ap().opt()])

            # -- SA V (2 token chunks of 128), rinv-scaled before the gather --
            wvt = wkv.tile([128, DCH, KVD], BF, tag="wkv", name="wkv")
            nc.sync.dma_start(out=wvt[:],
                              in_=din[f'sa_wv_{i}'].ap().rearrange("(k p) n -> p k n", p=128))
            for j in range(2):
                v_ps = pst(128, KVD, "vproj", tag="m")
                for k in range(DCH):
                    nc.tensor.matmul(v_ps[:], lhsT=xb[k][:, 128 * j:128 * (j + 1)],
                                     rhs=wvt[:, k, :], start=(k == 0), stop=(k == DCH - 1))
                vraw = workA.tile([128, KVD], BF, tag="vraw", name="vraw")
                nc.vector.tensor_scalar(out=vraw[:], in0=v_ps[:],
                                        scalar1=rvT[j][:], scalar2=None, op0=MUL)
                nc.sync.dma_start(out=ag_in[i].ap()[T + 128 * j:T + 128 * (j + 1), :],
                                  in_=vraw[:])

            nc.gpsimd.collective_compute(
                "AllGather", mybir.AluOpType.bypass, replica_groups=GROUPS,
                ins=[ag_in[i].ap()[T:2 * T, :].opt()], outs=[ag_out_v[i].ap().opt()])

            # -- SA Q (8 chunks) --
            ssqq_ps = pst(128, T, "ssqq", tag="o")
            for bki in range(2):
                wt = wbig.tile([128, DCH, 512], BF, tag="wbig", name="wbig")
                nc.sync.dma_start(
                    out=wt[:],
                    in_=din[f'sa_wq_{i}'].ap().rearrange("(k p) n -> p k n", p=128)
                    [:, :, bki * 512:(bki + 1) * 512])
                for jj in range(4):
                    mt = bki * 4 + jj
                    q_ps = pst(128, T, "qproj")
                    for k in range(DCH):
                        nc.tensor.matmul(q_ps[:], lhsT=wt[:, k, 128 * jj:128 * (jj + 1)],
                                         rhs=xb[k][:], start=(k == 0), stop=(k == DCH - 1))
                    qraw = workA.tile([128, T], BF, tag="qraw", name="qraw")
                    nc.scalar.activation(qraw[:], q_ps[:], Copy)
                    qsw_ps = pst(128, T, "qsw", tag="m")
                    nc.tensor.matmul(qsw_ps[:], lhsT=pswap[:], rhs=qraw[:],
                                     start=True, stop=True)
                    sqq = workA.tile([128, T], F16, tag="sqq", name="sqq")
                    nc.scalar.activation(sqq[:], q_ps[:], Square)
                    nc.tensor.matmul(ssqq_ps[:], lhsT=bdall[:, mt, :],
                                     rhs=sqq[:], start=(mt == 0), stop=(mt == 7))
                    nc.vector.tensor_tensor(aq[mt][:], q_ps[:], rtq_c[:], MUL)
                    qsw_m = workA.tile([128, T], FP, tag="qswm", name="qswm")
                    nc.vector.tensor_tensor(qsw_m[:], qsw_ps[:], rtq_s[:], MUL)
                    nc.vector.tensor_tensor(aq[mt][:], aq[mt][:], qsw_m[:], ADD)
            qsc = batch_norm_scale(ssqq_ps, 16, rv2=rv2)
            for mt in range(DCH):
                rbch = pair_bcast(qsc, mt, rbc=rbc)
                nc.vector.tensor_tensor(qf[mt][:], aq[mt][:], rbch[:], MUL)

            # -- CA K (2 chunks over enc, width TK) --
            ksc_t = workB.tile([128, 2], FP, tag="ksc", name="ksc")
            nc.sync.dma_start(out=ksc_t[:],
                              in_=din[f'ca_kscale_{i}'].ap().rearrange("(t p) o -> p (t o)", p=128))
            wkt2 = wkv.tile([128, DCH, KVD], BF, tag="wkv", name="wkv")
            nc.sync.dma_start(out=wkt2[:],
                              in_=din[f'ca_wk_{i}'].ap().rearrange("(k p) n -> p k n", p=128))
            ssqc_ps = pst(128, TK, "ssqc", tag="o")
            ckraw = []
            for t in range(2):
                k_ps = pst(128, TK, "ckproj")
                for k in range(DCH):
                    nc.tensor.matmul(k_ps[:], lhsT=wkt2[:, k, 128 * t:128 * (t + 1)],
                                     rhs=encb[k][:], start=(k == 0), stop=(k == DCH - 1))
                kr = aqp.tile([128, TK], BF, tag=f"ckr{t}", name=f"ckr{t}")
                nc.scalar.activation(kr[:], k_ps[:], Copy)
                ckraw.append(kr)
                sqc = workA.tile([128, TK], F16, tag="sqc", name="sqc")
                nc.scalar.activation(sqc[:], k_ps[:], Square)
                nc.tensor.matmul(ssqc_ps[:], lhsT=bdall[:, t, :],
                                 rhs=sqc[:], start=(t == 0), stop=(t == 1))
            crr = batch_norm_scale(ssqc_ps, 4, width=TK)
            for t in range(2):
                rbch = pair_bcast(crr, t, nrows=4, width=TK)
                kh = workA.tile([128, TK], FP, tag="khca", name="khca")
                nc.vector.tensor_tensor(kh[:], ckraw[t][:], rbch[:], MUL)
                ckp = workB.tile([128, TK], BF, tag="ckp", name="ckp")
                nc.vector.tensor_scalar(
                    out=ckp[:], in0=kh[:],
                    scalar1=ksc_t[:, t:t + 1], scalar2=None, op0=MUL)
                for half in range(2):
                    kv = 2 * t + half
                    for dd in range(2):
                        nc.scalar.dma_start(out=kdca[kv][64 * dd:64 * (dd + 1), :],
                                            in_=ckp[64 * half:64 * (half + 1), :])

            # -- CA V (4 chunks of 128 enc tokens) --
            wvt2 = wkv.tile([128, DCH, KVD], BF, tag="wkv", name="wkv")
            nc.sync.dma_start(out=wvt2[:],
                              in_=din[f'ca_wv_{i}'].ap().rearrange("(k p) n -> p k n", p=128))
            for kv in range(KVH):
                nc.vector.memset(cv[kv][:, :, 64:65], 1.0)
                nc.vector.memset(cv[kv][:, :, 65:128], 0.0)
            for j in range(TP):
                v_ps = pst(128, KVD, "vproj", tag="m")
                for k in range(DCH):
                    nc.tensor.matmul(v_ps[:], lhsT=encb[k][:, 128 * j:128 * (j + 1)],
                                     rhs=wvt2[:, k, :], start=(k == 0), stop=(k == DCH - 1))
                for kv in range(KVH):
                    nc.scalar.activation(cv[kv][:, j, 0:64],
                                         v_ps[:, 64 * kv:64 * (kv + 1)], Copy)

            if DBG and i == 0:
                nc.sync.dma_start(out=dbg['tap_rbc'].ap(), in_=rbc[:])
                nc.sync.dma_start(out=dbg['tap_xb'].ap(), in_=xb[0][:])
                nc.sync.dma_start(out=dbg['tap_qf'].ap(), in_=qf[0][:])
            sa_attention(i)
            if DBG and i == 0:
                nc.sync.dma_start(out=dbg['tap_ao'].ap(), in_=ao[0][:])
            stream_out_proj(f'sa_wo_{i}')
            if DBG and i == 0:
                nc.sync.dma_start(out=dbg['tap_x1'].ap(), in_=x[0][:])

            # ---- LN2 (deferred) + CA ----
            rbc, rv2 = ln_prep()

            # -- CA Q (8 chunks, no rope) --
            ssqq_ps = pst(128, T, "ssqq", tag="o")
            qrawb = []
            for bki in range(2):
                wt = wbig.tile([128, DCH, 512], BF, tag="wbig", name="wbig")
                nc.sync.dma_start(
                    out=wt[:],
                    in_=din[f'ca_wq_{i}'].ap().rearrange("(k p) n -> p k n", p=128)
                    [:, :, bki * 512:(bki + 1) * 512])
                for jj in range(4):
                    mt = bki * 4 + jj
                    q_ps = pst(128, T, "qproj")
                    for k in range(DCH):
                        nc.tensor.matmul(q_ps[:], lhsT=wt[:, k, 128 * jj:128 * (jj + 1)],
                                         rhs=xb[k][:], start=(k == 0), stop=(k == DCH - 1))
                    qr = aqp.tile([128, T], BF, tag=f"cqr{mt}", name=f"cqr{mt}")
                    nc.scalar.activation(qr[:], q_ps[:], Copy)
                    qrawb.append(qr)
                    sqq = workA.tile([128, T], F16, tag="sqq", name="sqq")
                    nc.scalar.activation(sqq[:], q_ps[:], Square)
                    nc.tensor.matmul(ssqq_ps[:], lhsT=bdall[:, mt, :],
                                     rhs=sqq[:], start=(mt == 0), stop=(mt == 7))
            qsc = batch_norm_scale(ssqq_ps, 16, rv2=rv2)
            for mt in range(DCH):
                rbch = pair_bcast(qsc, mt, rbc=rbc)
                nc.vector.tensor_tensor(qf[mt][:], qrawb[mt][:], rbch[:], MUL)

            ca_attention(i)
            stream_out_proj(f'ca_wo_{i}')

            # ---- LN3 (deferred) + FFN ----
            rbc, rv2 = ln_prep()

            NF = F // 512
            prods = []
            for fb in range(NF):
                wgt = wbig.tile([128, DCH, 512], BF, tag="wbig", name="wbig")
                nc.sync.dma_start(
                    out=wgt[:],
                    in_=din[f'ffn_wg_{i}'].ap().rearrange("(k p) n -> p k n", p=128)
                    [:, :, fb * 512:(fb + 1) * 512])
                wut = wbig.tile([128, DCH, 512], BF, tag="wbig", name="wbig")
                nc.sync.dma_start(
                    out=wut[:],
                    in_=din[f'ffn_wu_{i}'].ap().rearrange("(k p) n -> p k n", p=128)
                    [:, :, fb * 512:(fb + 1) * 512])
                for hf in range(2):
                    gu = []
                    for which, wt in (('g', wgt), ('u', wut)):
                        g_ps = pst(128, 512, "s")
                        for jj in range(2):
                            j = 2 * hf + jj
                            for k in range(DCH):
                                nc.tensor.matmul(
                                    g_ps[:, 256 * jj:256 * (jj + 1)],
                                    lhsT=wt[:, k, 128 * j:128 * (j + 1)],
                                    rhs=xb[k][:], start=(k == 0), stop=(k == DCH - 1))
                        g_sb = ffnp.tile([128, 512], BF, tag=f"relu{which}", name=f"relu{which}")
                        if which == 'g':
                            nc.scalar.activation(g_sb[:], g_ps[:], Relu)
                        else:
                            nc.vector.tensor_scalar(out=g_sb[:], in0=g_ps[:],
                                                    scalar1=0.0, scalar2=None,
                                                    op0=AluOpType.max)
                        gu.append(g_sb)
                    pr = ffnp.tile([128, 512], BF, tag="prod", name="prod", bufs=16)
                    nc.vector.tensor_tensor(pr[:], gu[0][:], gu[1][:], MUL)
                    prods.append(pr)
            # down-proj in 4 column passes: each m-pair accumulates its full
            # F-contraction (32 chunk matmuls per half) in one psum bank, so
            # the per-fb partial-sum adds disappear entirely
            NCH = F // 128
            for m in range(DCH):
                wdp = wbig.tile([128, NCH, 128], BF, tag="wbig", name="wdp")
                nc.sync.dma_start(
                    out=wdp[:],
                    in_=din[f'ffn_wd_{i}'].ap().rearrange("(k p) n -> p k n", p=128)
                    [:, :, 128 * m:128 * (m + 1)])
                yp = pst(128, T, "yp", tag="o")
                for ch in range(NCH):
                    pi = 2 * (ch // 4) + (ch % 4) // 2
                    nc.tensor.matmul(
                        yp[:],
                        lhsT=wdp[:, ch, :],
                        rhs=prods[pi][:, 256 * (ch % 2):256 * (ch % 2 + 1)],
                        start=(ch == 0), stop=(ch == NCH - 1))
                nc.vector.tensor_tensor(y_sb[m][:], yp[:], rv2[:], MUL)
                nc.vector.tensor_tensor(x[m][:], x[m][:], y_sb[m][:], ADD)

        # ---- final norm + output ----
        ssf = pst(128, T, "ssf", tag="o")
        for m in range(DCH):
            sq = workA.tile([128, T], F16, tag="sq", name="sq")
            nc.vector.tensor_tensor(sq[:], x[m][:], x[m][:], MUL)
            nc.tensor.matmul(ssf[:], lhsT=ones[:], rhs=sq[:],
                             start=(m == 0), stop=(m == DCH - 1))
        lnf = workB.tile([128, T], FP, tag="lnf", name="lnf")
        nc.scalar.activation(lnf[:], ssf[:], Ln, bias=eps_t[:], scale=1.0 / D)
        rbcf = workA.tile([128, T], FP, tag="rbcf", name="rbcf")
        nc.scalar.activation(rbcf[:], lnf[:], Exp, scale=-0.5)
        for m in range(DCH):
            ot = workB.tile([128, T], FP, tag="otile", name="otile", bufs=2)
            nc.vector.tensor_tensor(ot[:], x[m][:], rbcf[:], MUL)
            nc.vector.tensor_scalar(out=ot[:], in0=ot[:],
                                    scalar1=fscale[:, m:m + 1], scalar2=None, op0=MUL)
            nc.sync.dma_start(out=out_dram.ap()[128 * m:128 * (m + 1), :], in_=ot[:])

    nc.compile()
    return nc


def _get_program():
    global _PROG
    if _PROG is None:
        _PROG = _build_program()
    return _PROG


def kernel(**inputs):
    from concourse import bass_utils
    host, per_core = host_prepare(inputs)
    nc = _get_program()
    in_maps = []
    for c in range(NCORES):
        m = dict(per_core[c])
        for k, v in host.items():
            if k.startswith('rope'):
                continue  # per-core sliced versions already present
            m[k] = v
        in_maps.append(m)
    res = bass_utils.run_bass_kernel_spmd(nc, in_maps, list(range(NCORES)))
    out = np.empty((B, TQ, D), np.float32)
    for c in range(NCORES):
        grp, r = c // TP, c % TP
        out[grp, r::TP] = res.results[c]['outT'].T
    return out



# revision 49
# speedup vs baseline: 1.0906x; 1.0906x over previous
"""Self-contained Trainium2 Bass kernel for nn_EncoderDecoderTransformer_90941637525663.

Sequence-parallel over 8 NeuronCores (2 batch groups x 4 token shards), with
INTERLEAVED token assignment (core r owns tokens t = 4j + r): causal
visibility becomes structurally uniform across cores, so each head-pair
processes 12 score blocks instead of 20 (full blocks need no mask; the four
diagonal blocks use a per-core triangular mask loaded as input). Activations
live transposed in SBUF (feature dim on partitions, tokens on free dim).
Per layer, K (normalized + roped) and V (pre-scaled by sender 1/rms) are
exchanged with AllGathers split into a K gather (posted early, feeds scores)
and a V gather (overlaps the first score/exp wave). All heavy matmuls run in
bf16; helper selector/accumulator weights (ones/bdall/sel16/squares/scales)
are fp16 so matmuls stay on the fast full-width weight path. All rms factors
use exp(-0.5*ln(mean+eps)) so every activation lives in one ACT table set
(one table load total). Softmax row-sum division is batched per head pair:
vector reciprocal + gpsimd partition-broadcast. LN scales are folded into
consumer weights host-side; deferred-LN per-head rms folds rinv in via rv2.
"""
import sys
sys.path.insert(0, '/opt/trn_rl_repo')
import numpy as np

B, TQ, TK, D, H, KVH, L, F = 2, 1024, 512, 1024, 16, 4, 2, 4096
HD, KVD = 64, 256
EPS = 1e-6
NCORES, TP = 8, 4
T = TQ // TP           # 256 tokens per core (interleaved: t = 4j + r)
DCH = D // 128         # 8 feature chunks
NEG = -3.0e38
AGR = 2 * T            # ag rows: [0,T)=k (normalized), [T,2T)=v (rinv-scaled)


def _bf16():
    import ml_dtypes
    return ml_dtypes.bfloat16


def _rope_tables(Tlen, hd, theta=10000.0):
    freqs = 1.0 / theta ** (np.arange(0, hd, 2, dtype=np.float32) / hd)
    ang = np.outer(np.arange(Tlen, dtype=np.float32), freqs)
    return np.cos(ang).astype(np.float32), np.sin(ang).astype(np.float32)


def host_prepare(inputs):
    """Returns (host, per_core): folded shared arrays + per-core arrays."""
    bf16 = _bf16()
    inputs = {k: np.ascontiguousarray(np.asarray(v, dtype=np.float32))
              for k, v in inputs.items()}
    cos_f, sin_f = _rope_tables(TQ, HD)       # [TQ, 32]

    def bf(a):
        return np.ascontiguousarray(a.astype(bf16))

    fp16 = np.float16

    host = {}
    for i in range(L):
        ln1 = (1.0 + inputs['ln1_s'][i])[:, None]
        ln2 = (1.0 + inputs['ln2_s'][i])[:, None]
        ln3 = (1.0 + inputs['ln3_s'][i])[:, None]
        host[f'sa_wq_{i}'] = bf(ln1 * inputs['sa_wq'][i])
        host[f'sa_wk_{i}'] = bf(ln1 * inputs['sa_wk'][i])
        host[f'sa_wv_{i}'] = bf(ln1 * inputs['sa_wv'][i])
        host[f'sa_wo_{i}'] = bf(inputs['sa_wo'][i])
        host[f'ca_wq_{i}'] = bf(ln2 * inputs['ca_wq'][i])
        host[f'ca_wk_{i}'] = bf(inputs['ca_wk'][i])
        host[f'ca_wv_{i}'] = bf(inputs['ca_wv'][i])
        host[f'ca_wo_{i}'] = bf(inputs['ca_wo'][i])
        host[f'ffn_wg_{i}'] = bf(ln3 * inputs['ffn_wg'][i])
        host[f'ffn_wu_{i}'] = bf(ln3 * inputs['ffn_wu'][i])
        host[f'ffn_wd_{i}'] = bf(inputs['ffn_wd'][i])
        # rope tables with (1+qn)/(1+kn) head-dim scales folded; [128, TQ]
        for which, dvec in [('q', inputs['sa_qn'][i]), ('k', inputs['sa_kn'][i])]:
            d1, d2 = 1.0 + dvec[:32], 1.0 + dvec[32:]
            C = np.concatenate([d1[:, None] * cos_f.T, d2[:, None] * cos_f.T], 0)
            S = np.concatenate([-d2[:, None] * sin_f.T, d1[:, None] * sin_f.T], 0)
            host[f'rope{which}_c_{i}'] = np.ascontiguousarray(np.concatenate([C, C], 0))
            host[f'rope{which}_s_{i}'] = np.ascontiguousarray(np.concatenate([S, S], 0))
        sc = ((1.0 + inputs['ca_qn'][i]) * (1.0 + inputs['ca_kn'][i])).astype(np.float32)
        host[f'ca_kscale_{i}'] = np.tile(sc, KVH)[:, None].copy()   # [256, 1]
    host['final_scale'] = (1.0 + inputs['final_s'])[:, None].copy()  # [D, 1]
    # bdall[:, mt, :]: [128,128] stationary (fp16, M padded to 128 for fast
    # weight load) accumulating chunk mt's per-head-pair row sums into rows
    # 2mt/2mt+1 of a [128,T] psum (other rows zero)
    bdall = np.zeros((128, 8, 128), fp16)
    for mt in range(8):
        bdall[0:64, mt, 2 * mt] = 1.0
        bdall[64:128, mt, 2 * mt + 1] = 1.0
    host['bdall'] = bdall
    # sel16[0:n, mt, :]: [n,128] stationary (fp16) broadcasting rows 2mt/2mt+1
    # of an [n,T] scale tile to the 64-row head halves of a [128,T] output
    sel16 = np.zeros((16, 8, 128), fp16)
    for mt in range(8):
        sel16[2 * mt, mt, 0:64] = 1.0
        sel16[2 * mt + 1, mt, 64:128] = 1.0
    host['sel16'] = sel16
    # rope half-swap permutation (within each 64-row head block): [128, 128]
    psw = np.zeros((128, 128), np.float32)
    for base in (0, 64):
        for r in range(32):
            psw[base + 32 + r, base + r] = 1.0      # out row base+r <- in row base+32+r
            psw[base + r, base + 32 + r] = 1.0      # out row base+32+r <- in row base+r
    host['pswap'] = bf(psw)

    per_core = []
    kl = np.arange(128)[:, None]
    ql = np.arange(128)[None, :]
    tri_inc = np.where(kl <= ql, 0.0, NEG).astype(np.float32)
    tri_exc = np.where(kl < ql, 0.0, NEG).astype(np.float32)
    for c in range(NCORES):
        grp, r = c // TP, c % TP
        tok = slice(r, None, TP)   # interleaved tokens: 4j + r
        pc = {
            'xT': np.ascontiguousarray(inputs['x'][grp].T[:, tok]),
            'encTb': bf(inputs['encoder_out'][grp].T),
        }
        # diagonal-chunk additive mask vs source core g: k-token 4p+g visible
        # to q-token 4j+r within the same 128-block iff p<=j (g<=r) / p<j (g>r)
        dm = np.empty((128, TP, 128), np.float32)
        for g in range(TP):
            dm[:, g, :] = tri_inc if g <= r else tri_exc
        pc['dmask'] = np.ascontiguousarray(dm)
        for i in range(L):
            for nm in ('ropeq_c', 'ropeq_s', 'ropek_c', 'ropek_s'):
                pc[f'{nm}_{i}'] = np.ascontiguousarray(host[f'{nm}_{i}'][:, tok])
        per_core.append(pc)
    return host, per_core


_PROG = None
DBG = False


def _build_program(nlayers=L):
    import concourse.bass as bass
    import concourse.tile as tile
    from concourse import bacc, mybir
    from concourse.alu_op_type import AluOpType
    from contextlib import ExitStack

    R = mybir.dt.float32r
    FP = mybir.dt.float32
    BF = mybir.dt.bfloat16
    F16 = mybir.dt.float16
    Exp = mybir.ActivationFunctionType.Exp
    Ln = mybir.ActivationFunctionType.Ln
    Relu = mybir.ActivationFunctionType.Relu
    Square = mybir.ActivationFunctionType.Square
    Copy = mybir.ActivationFunctionType.Copy

    class _Bacc(bacc.Bacc):
        def insert_act_table_loads(self):
            # Force exp/ln (this kernel's only table-bound activations) to
            # resolve to the combined natural_log_exp_and_others set: the
            # default first-match selection alternates exp_and_others /
            # natural_log and pays a ~1.3us table load per switch.
            import bass_rust as _bass_rust
            has_activation = any(
                isinstance(ins, mybir.InstActivation)
                for b in self.main_func.blocks
                for ins in b.instructions
            )
            if not has_activation:
                return
            from concourse.hw_specs import get_activation_tables
            tables = list(get_activation_tables(self.m.arch).items())
            combined = next(i for i, (nm, _) in enumerate(tables)
                            if nm == 'natural_log_exp_and_others')
            tables = [(nm, s if i == combined else set())
                      for i, (nm, s) in enumerate(tables)]
            _bass_rust.insert_act_table_loads(self, tables)

    nc = _Bacc("TRN2", target_bir_lowering=False, debug=False,
               num_devices=NCORES)

    din = {}
    def dri(name, shape, dt):
        din[name] = nc.dram_tensor(name, list(shape), dt, kind="ExternalInput")

    dri('xT', (D, T), FP)
    dri('encTb', (D, TK), BF)
    dri('dmask', (128, TP, 128), FP)
    dri('final_scale', (D, 1), FP)
    dri('bdall', (128, 8, 128), F16)
    dri('sel16', (16, 8, 128), F16)
    dri('pswap', (128, 128), BF)
    for i in range(nlayers):
        dri(f'sa_wq_{i}', (D, D), BF); dri(f'sa_wk_{i}', (D, KVD), BF)
        dri(f'sa_wv_{i}', (D, KVD), BF); dri(f'sa_wo_{i}', (D, D), BF)
        dri(f'ca_wq_{i}', (D, D), BF); dri(f'ca_wk_{i}', (D, KVD), BF)
        dri(f'ca_wv_{i}', (D, KVD), BF); dri(f'ca_wo_{i}', (D, D), BF)
        dri(f'ffn_wg_{i}', (D, F), BF); dri(f'ffn_wu_{i}', (D, F), BF)
        dri(f'ffn_wd_{i}', (F, D), BF)
        for nm in ('ropeq_c', 'ropeq_s', 'ropek_c', 'ropek_s'):
            dri(f'{nm}_{i}', (128, T), FP)
        dri(f'ca_kscale_{i}', (KVD, 1), FP)
    out_dram = nc.dram_tensor('outT', [D, T], FP, kind="ExternalOutput")
    dbg = {}
    if DBG:
        for nm, shp, dt in [('tap_rbc', [128, T], FP), ('tap_xb', [128, T], BF),
                            ('tap_qsc', [16, T], FP), ('tap_qf', [128, T], BF),
                            ('tap_kd', [128, T], BF), ('tap_vown', [128, 256], BF),
                            ('tap_kfull', [128, TQ], BF), ('tap_vfull', [128, TP * 2 * 128], BF),
                            ('tap_rvg', [128, TP * 2], BF), ('tap_ao', [128, T], BF),
                            ('tap_x1', [128, T], FP)]:
            dbg[nm] = nc.dram_tensor(nm, shp, dt, kind="ExternalOutput")
    ag_in = [nc.dram_tensor(f'ag_in_{i}', [AGR, T], BF) for i in range(nlayers)]
    ag_out_k = [nc.dram_tensor(f'ag_out_k_{i}', [T * TP, T], BF) for i in range(nlayers)]
    ag_out_v = [nc.dram_tensor(f'ag_out_v_{i}', [T * TP, T], BF) for i in range(nlayers)]
    agw_in = nc.dram_tensor('agw_in', [1, 128], BF)
    agw_out = nc.dram_tensor('agw_out', [TP, 128], BF)
    GROUPS = [[0, 1, 2, 3], [4, 5, 6, 7]]

    with nc.allow_low_precision(reason="bf16 pipeline"), \
            tile.TileContext(nc) as tc, ExitStack() as ctx:
        consts = ctx.enter_context(tc.tile_pool(name="consts", bufs=1))
        state = ctx.enter_context(tc.tile_pool(name="state", bufs=1))
        aqp = ctx.enter_context(tc.tile_pool(name="aqp", bufs=1))
        kvf = ctx.enter_context(tc.tile_pool(name="kvf", bufs=2))
        wbig = ctx.enter_context(tc.tile_pool(name="wbig", bufs=3))
        wkv = ctx.enter_context(tc.tile_pool(name="wkv", bufs=2))
        workA = ctx.enter_context(tc.tile_pool(name="workA", bufs=2))
        workB = ctx.enter_context(tc.tile_pool(name="workB", bufs=2))
        psb = ctx.enter_context(tc.tile_pool(name="psb", bufs=4))
        ffnp = ctx.enter_context(tc.tile_pool(name="ffnp", bufs=2))
        ropep = ctx.enter_context(tc.tile_pool(name="ropep", bufs=2))
        ps = ctx.enter_context(tc.tile_pool(name="ps", bufs=8, space="PSUM"))

        def pst(p_, f_, name, tag="s"):
            bufs = {"s": 3, "o": 2, "m": 3}[tag]
            return ps.tile([p_, f_], FP, tag=f"ps_{tag}", name=name, bufs=bufs)

        MUL, ADD = AluOpType.mult, AluOpType.add

        # ---- warm-up collective: the first collective's entry barrier
        # absorbs inter-core launch skew; run it on a tiny buffer up front so
        # the wait overlaps constant/weight loads and the pre-AG compute ----
        warm = consts.tile([1, 128], BF, tag="warm", name="warm")
        nc.vector.memset(warm[:], 0.0)
        nc.sync.dma_start(out=agw_in.ap(), in_=warm[:])
        nc.gpsimd.collective_compute(
            "AllGather", mybir.AluOpType.bypass, replica_groups=GROUPS,
            ins=[agw_in.ap().opt()], outs=[agw_out.ap().opt()])

        # ---- constants ----
        ones = consts.tile([128, 128], F16, tag="ones", name="ones")
        nc.vector.memset(ones[:], 1.0)
        bdall = consts.tile([128, 8, 128], F16, tag="bdall", name="bdall")
        nc.sync.dma_start(out=bdall[:], in_=din['bdall'].ap())
        sel16 = consts.tile([16, 8, 128], F16, tag="sel16", name="sel16")
        nc.sync.dma_start(out=sel16[:], in_=din['sel16'].ap())
        eps_t = consts.tile([128, 1], FP, tag="eps", name="eps")
        nc.vector.memset(eps_t[:], EPS)
        id1 = consts.tile([1, 1], FP, tag="id1", name="id1")
        nc.vector.memset(id1[:], 1.0)
        pswap = consts.tile([128, 128], BF, tag="pswap", name="pswap")
        nc.sync.dma_start(out=pswap[:], in_=din['pswap'].ap())
        dmask = consts.tile([128, TP, 128], FP, tag="dmask", name="dmask")
        nc.sync.dma_start(out=dmask[:], in_=din['dmask'].ap())
        fscale = consts.tile([128, DCH], FP, tag="fscale", name="fscale")
        nc.sync.dma_start(out=fscale[:],
                          in_=din['final_scale'].ap().rearrange("(k p) o -> p (k o)", p=128))

        # ---- persistent state ----
        x = [state.tile([128, T], FP, tag=f"x{m}", name=f"x{m}") for m in range(DCH)]
        for m in range(DCH):
            nc.sync.dma_start(out=x[m][:], in_=din['xT'].ap()[128 * m:128 * (m + 1), :])
        xb = [state.tile([128, T], BF, tag=f"xb{m}", name=f"xb{m}") for m in range(DCH)]
        encb = [state.tile([128, TK], BF, tag=f"enc{m}", name=f"enc{m}") for m in range(DCH)]
        for m in range(DCH):
            nc.sync.dma_start(out=encb[m][:], in_=din['encTb'].ap()[128 * m:128 * (m + 1), :])
        ao = [state.tile([128, T], BF, tag=f"ao{m}", name=f"ao{m}") for m in range(DCH)]
        qf = [state.tile([128, T], BF, tag=f"qf{t}", name=f"qf{t}") for t in range(H // 2)]
        kdca = [state.tile([128, TK], BF, tag=f"kdca{k}", name=f"kdca{k}") for k in range(KVH)]
        cv = [state.tile([128, TP, 128], BF, tag=f"cv{k}", name=f"cv{k}") for k in range(KVH)]
        y_sb = [state.tile([128, T], FP, tag=f"ysb{m}", name=f"ysb{m}") for m in range(DCH)]
        aq = y_sb

        # ---------------- helpers ----------------
        def ln_prep(tagp=""):
            """rms over D partitions of fp32 x -> (rbc [128,T] FP broadcast of
            1/rms, rv2 [128,T] FP = rbc^2). Also refreshes xb (bf16 cast of x).
            Uses exp(-0.5*ln(mean+eps)) on full 128 rows so no partition
            broadcast is needed (ACT cost is free-dim bound)."""
            ss = pst(128, T, "ss", tag="o")
            for m in range(DCH):
                sq = workA.tile([128, T], F16, tag="sq", name="sq")
                nc.vector.tensor_tensor(sq[:], x[m][:], x[m][:], MUL)
                nc.tensor.matmul(ss[:], lhsT=ones[:], rhs=sq[:],
                                 start=(m == 0), stop=(m == DCH - 1))
                nc.vector.tensor_copy(xb[m][:], x[m][:])
            lnv = workA.tile([128, T], FP, tag="lnv" + tagp, name="lnv")
            nc.scalar.activation(lnv[:], ss[:], Ln, bias=eps_t[:], scale=1.0 / D)
            rbc = workA.tile([128, T], FP, tag="rbc" + tagp, name="rbc")
            nc.scalar.activation(rbc[:], lnv[:], Exp, scale=-0.5)
            rv2 = workA.tile([128, T], FP, tag="rv2" + tagp, name="rv2")
            nc.vector.tensor_tensor(rv2[:], rbc[:], rbc[:], MUL)
            return rbc, rv2

        def batch_norm_scale(ssq_ps, nrows, rv2=None, width=T):
            """Per-head-pair raw sums [nrows, width] psum -> 1/rms scale (F16)
            via exp(-0.5*ln(mean+eps)). With rv2: deferred-LN variant (sums
            scaled by rinv^2 first)."""
            if rv2 is not None:
                tk = workA.tile([nrows, width], FP, tag="bnt", name="bnt",
                                padded_shape=[16, TK])
                nc.vector.tensor_tensor(tk[:], ssq_ps[0:nrows, 0:width],
                                        rv2[0:nrows, :width], MUL)
                src = tk[:]
            else:
                src = ssq_ps[0:nrows, 0:width]
            sr = workB.tile([nrows, width], FP, tag="bns", name="bns",
                            padded_shape=[16, TK])
            nc.scalar.activation(sr[:], src, Ln, bias=eps_t[0:nrows, :], scale=1.0 / HD)
            rr = workB.tile([nrows, width], F16, tag="bnr", name="bnr",
                            padded_shape=[16, TK])
            nc.scalar.activation(rr[:], sr[:], Exp, scale=-0.5)
            return rr

        def pair_bcast(scale_tile, t, nrows=16, width=T, rbc=None):
            """Broadcast rows 2t/2t+1 of an [nrows, width] F16 scale tile to the
            64-row halves of a [128, width] FP tile via PE selector matmul.
            If rbc given, fold the per-token rinv into the drain."""
            bc_ps = pst(128, width, "bch", tag="m")
            nc.tensor.matmul(bc_ps[:], lhsT=sel16[0:nrows, t, :],
                             rhs=scale_tile[:],
                             start=True, stop=True)
            rbch = workA.tile([128, width], FP, tag="rbch", name="rbch",
                              padded_shape=[128, TK])
            if rbc is not None:
                nc.vector.tensor_tensor(rbch[:], bc_ps[:], rbc[:, :width], MUL)
            else:
                nc.scalar.activation(rbch[:], bc_ps[:], Copy)
            return rbch

        def norm_pair(qt, o2):
            """Divide both heads' outputs by their softmax rowsums (row 64 of
            each o2): recip on vector, partition-broadcast on the otherwise
            idle gpsimd, final muls on vector."""
            rs2 = workB.tile([1, 2, T], FP, tag="rs2", name="rs2", bufs=3)
            for h in range(2):
                nc.scalar.activation(rs2[0:1, h, :], o2[h][64:65, :], Copy)
            rr2 = workB.tile([1, 2, T], FP, tag="rr2", name="rr2", bufs=3)
            nc.vector.reciprocal_approx_fast(rr2[:], rs2[:])
            bc_sb = workB.tile([128, 2, T], FP, tag="bc_sb", name="bc_sb", bufs=3)
            nc.gpsimd.partition_broadcast(bc_sb[:], rr2[:])
            for h, par in ((0, 0), (1, 64)):
                nc.vector.tensor_tensor(ao[qt][par:par + 64, :],
                                        o2[h][0:64, :],
                                        bc_sb[par:par + 64, h, :], MUL)

        def sa_attention(i):
            agk = ag_out_k[i].ap()
            agv = ag_out_v[i].ap()
            for kv in range(KVH):
                kga = kvf.tile([128, TP, T], BF, tag="kfull", name="kfull")
                src = bass.AP(tensor=agk.tensor, offset=(64 * kv) * T,
                              ap=[[T, 64], [T * T, TP], [1, T]])
                for dd in range(2):
                    nc.sync.dma_start(out=kga[64 * dd:64 * (dd + 1), :, :], in_=src)
                vga = kvf.tile([128, TP, 2, 128], BF, tag="vfull", name="vfull")
                nc.vector.memset(vga[:, :, :, 64:65], 1.0)
                nc.vector.memset(vga[:, :, :, 65:128], 0.0)
                for cb in range(2):
                    vap = bass.AP(tensor=agv.tensor,
                                  offset=128 * T * cb + 64 * kv,
                                  ap=[[T, 128], [T * T, TP], [1, 64]])
                    nc.sync.dma_start(out=vga[:, :, cb, 0:64], in_=vap)
                for pp in range(2):
                    qt = 2 * kv + pp
                    o2 = [pst(128, T, f"o{h}", tag="o") for h in range(2)]
                    mi = 0
                    # C=0 chunks [128k x 256q]: left half (vs J0) is the
                    # diagonal (masked), right half (vs J1) fully visible
                    for b in range(2):
                        s2t = [pst(128, 512, "s") for _ in range(2)]
                        for gg in range(2):
                            g = 2 * b + gg
                            for h, par in ((0, 0), (1, 64)):
                                nc.tensor.matmul(
                                    s2t[h][:, 256 * gg:256 * (gg + 1)],
                                    lhsT=kga[par:par + 64, g, 0:128],
                                    rhs=qf[qt][par:par + 64, :],
                                    start=True, stop=True)
                        p2t = []
                        for h in range(2):
                            sv = s2t[h][:].rearrange("p (c t) -> p c t", c=2)
                            nc.vector.tensor_tensor(
                                sv[:, :, 0:128], sv[:, :, 0:128],
                                dmask[:, 2 * b:2 * b + 2, :], ADD)
                            p_sb = psb.tile([128, 2, T], BF, tag="p_sb", name="p_sb")
                            nc.scalar.activation(p_sb[:], sv, Exp, scale=0.125)
                            p2t.append(p_sb)
                        for gg in range(2):
                            g = 2 * b + gg
                            for h in range(2):
                                nc.tensor.matmul(o2[h][:], lhsT=vga[:, g, 0, :],
                                                 rhs=p2t[h][:, gg, :],
                                                 start=(mi == 0), stop=False,
                                                 skip_group_check=True)
                            mi += 1
                    # C=1 chunks [128k x 128q]: all diagonal, right half only
                    s2t = [pst(128, 512, "s") for _ in range(2)]
                    for h, par in ((0, 0), (1, 64)):
                        for g in range(TP):
                            nc.tensor.matmul(
                                s2t[h][:, 128 * g:128 * (g + 1)],
                                lhsT=kga[par:par + 64, g, 128:256],
                                rhs=qf[qt][par:par + 64, 128:256],
                                start=True, stop=True)
                    p2t = []
                    for h in range(2):
                        sv = s2t[h][:].rearrange("p (g t) -> p g t", g=TP)
                        nc.vector.tensor_tensor(sv, sv, dmask[:], ADD)
                        p_sb = psb.tile([128, TP, 128], BF, tag="p_sb", name="p_sb")
                        nc.scalar.activation(p_sb[:], sv, Exp, scale=0.125)
                        p2t.append(p_sb)
                    for g in range(TP):
                        for h in range(2):
                            nc.tensor.matmul(o2[h][:, 128:256], lhsT=vga[:, g, 1, :],
                                             rhs=p2t[h][:, g, :],
                                             start=False, stop=(g == TP - 1),
                                             skip_group_check=True)
                    norm_pair(qt, o2)

        def ca_attention(i):
            nk_chunks = TK // 128
            for kv in range(KVH):
                ksrc = kdca[kv]
                vsrc = cv[kv]
                for pp in range(2):
                    # process both heads of pair tile qt together: their score
                    # matmuls use disjoint PE row groups (lhsT base 0 vs 64) and
                    # overlap when issued back-to-back
                    qt = 2 * kv + pp
                    o2 = [pst(128, T, f"o{h}", tag="o") for h in range(2)]
                    mi = 0
                    for half in range(nk_chunks // 2):
                        s2t = [pst(128, 512, "s") for _ in range(2)]
                        for cc in range(2):
                            c = 2 * half + cc
                            for h, par in ((0, 0), (1, 64)):
                                nc.tensor.matmul(
                                    s2t[h][:, 256 * cc:256 * (cc + 1)],
                                    lhsT=ksrc[par:par + 64, 128 * c:128 * (c + 1)],
                                    rhs=qf[qt][par:par + 64, :], start=True, stop=True)
                        p2t = []
                        for h in range(2):
                            p_sb = psb.tile([128, 2, T], BF, tag="p_sb", name="p_sb")
                            nc.scalar.activation(p_sb[:], s2t[h][:].rearrange(
                                "p (c t) -> p c t", c=2), Exp, scale=0.125)
                            p2t.append(p_sb)
                        for cc in range(2):
                            c = 2 * half + cc
                            for h in range(2):
                                nc.tensor.matmul(o2[h][:], lhsT=vsrc[:, c, :],
                                                 rhs=p2t[h][:, cc, :],
                                                 start=(mi == 0),
                                                 stop=(mi == nk_chunks - 1),
                                                 skip_group_check=True)
                            mi += 1
                    norm_pair(qt, o2)

        def stream_out_proj(w_name):
            for bki in range(2):
                wt = wbig.tile([128, DCH, 512], BF, tag="wbig", name="wbig")
                nc.sync.dma_start(
                    out=wt[:],
                    in_=din[w_name].ap().rearrange("(k p) n -> p k n", p=128)
                    [:, :, bki * 512:(bki + 1) * 512])
                for j in range(4):
                    m = bki * 4 + j
                    y_ps = pst(128, T, "proj", tag="m")
                    for k in range(DCH):
                        nc.tensor.matmul(y_ps[:], lhsT=wt[:, k, 128 * j:128 * (j + 1)],
                                         rhs=ao[k][:],
                                         start=(k == 0), stop=(k == DCH - 1))
                    nc.vector.tensor_tensor(x[m][:], x[m][:], y_ps[:], ADD)

        # ================= layers =================
        for i in range(nlayers):
            # ---- LN1 (deferred) + SA ----
            rbc, rv2 = ln_prep()
            rvT = []
            for j in range(2):
                tp_ps = pst(128, 1, "tp", tag="m")
                nc.tensor.transpose(tp_ps[:], rbc[0:1, 128 * j:128 * (j + 1)], id1[:])
                rv_b = workB.tile([128, 1], FP, tag=f"rvT{j}", name=f"rvT{j}")
                nc.scalar.activation(rv_b[:], tp_ps[:], Copy)
                rvT.append(rv_b)

            rtq_c = ropep.tile([128, T], FP, tag="rtc", name="rtc")
            nc.sync.dma_start(out=rtq_c[:], in_=din[f'ropeq_c_{i}'].ap())
            rtq_s = ropep.tile([128, T], FP, tag="rts", name="rts")
            nc.sync.dma_start(out=rtq_s[:], in_=din[f'ropeq_s_{i}'].ap())
            rtk_c = ropep.tile([128, T], FP, tag="rtc", name="rtc")
            nc.sync.dma_start(out=rtk_c[:], in_=din[f'ropek_c_{i}'].ap())
            rtk_s = ropep.tile([128, T], FP, tag="rts", name="rts")
            nc.sync.dma_start(out=rtk_s[:], in_=din[f'ropek_s_{i}'].ap())

            # -- SA K (2 chunks of 128 cols) --
            wkt = wkv.tile([128, DCH, KVD], BF, tag="wkv", name="wkv")
            nc.sync.dma_start(out=wkt[:],
                              in_=din[f'sa_wk_{i}'].ap().rearrange("(k p) n -> p k n", p=128))
            ssqk_ps = pst(128, T, "ssqk", tag="o")
            ka = []
            for t in range(2):
                k_ps = pst(128, T, "kproj")
                for k in range(DCH):
                    nc.tensor.matmul(k_ps[:], lhsT=wkt[:, k, 128 * t:128 * (t + 1)],
                                     rhs=xb[k][:], start=(k == 0), stop=(k == DCH - 1))
                kraw = workA.tile([128, T], BF, tag="kraw", name="kraw")
                nc.scalar.activation(kraw[:], k_ps[:], Copy)
                ksw_ps = pst(128, T, "ksw", tag="m")
                nc.tensor.matmul(ksw_ps[:], lhsT=pswap[:], rhs=kraw[:],
                                 start=True, stop=True)
                sqk = workA.tile([128, T], F16, tag="sqk", name="sqk")
                nc.scalar.activation(sqk[:], k_ps[:], Square)
                nc.tensor.matmul(ssqk_ps[:], lhsT=bdall[:, t, :],
                                 rhs=sqk[:], start=(t == 0), stop=(t == 1))
                kat = workB.tile([128, T], FP, tag=f"ka{t}", name=f"ka{t}")
                nc.vector.tensor_tensor(kat[:], k_ps[:], rtk_c[:], MUL)
                ksw_m = workA.tile([128, T], FP, tag="kswm", name="kswm")
                nc.vector.tensor_tensor(ksw_m[:], ksw_ps[:], rtk_s[:], MUL)
                nc.vector.tensor_tensor(kat[:], kat[:], ksw_m[:], ADD)
                ka.append(kat)
            ksc = batch_norm_scale(ssqk_ps, 4, rv2=rv2)
            for t in range(2):
                rbch = pair_bcast(ksc, t, nrows=4, rbc=rbc)
                kf = workA.tile([128, T], BF, tag="kf", name="kf")
                nc.vector.tensor_tensor(kf[:], ka[t][:], rbch[:], MUL)
                nc.sync.dma_start(out=ag_in[i].ap()[128 * t:128 * (t + 1), :],
                                  in_=kf[:])
            # gather K on its own (scores only need K; V follows in a second
            # collective that overlaps the first score/exp wave)
            nc.gpsimd.collective_compute(
                "AllGather", mybir.AluOpType.bypass, replica_groups=GROUPS,
                ins=[ag_in[i].ap()[0:T, :].opt()], outs=[ag_out_k[i].ap().opt()])

            # -- SA V (2 token chunks of 128), rinv-scaled before the gather --
            wvt = wkv.tile([128, DCH, KVD], BF, tag="wkv", name="wkv")
            nc.sync.dma_start(out=wvt[:],
                              in_=din[f'sa_wv_{i}'].ap().rearrange("(k p) n -> p k n", p=128))
            for j in range(2):
                v_ps = pst(128, KVD, "vproj", tag="m")
                for k in range(DCH):
                    nc.tensor.matmul(v_ps[:], lhsT=xb[k][:, 128 * j:128 * (j + 1)],
                                     rhs=wvt[:, k, :], start=(k == 0), stop=(k == DCH - 1))
                vraw = workA.tile([128, KVD], BF, tag="vraw", name="vraw")
                nc.vector.tensor_scalar(out=vraw[:], in0=v_ps[:],
                                        scalar1=rvT[j][:], scalar2=None, op0=MUL)
                nc.sync.dma_start(out=ag_in[i].ap()[T + 128 * j:T + 128 * (j + 1), :],
                                  in_=vraw[:])

            nc.gpsimd.collective_compute(
                "AllGather", mybir.AluOpType.bypass, replica_groups=GROUPS,
                ins=[ag_in[i].ap()[T:2 * T, :].opt()], outs=[ag_out_v[i].ap().opt()])

            # -- SA Q (8 chunks) --
            ssqq_ps = pst(128, T, "ssqq", tag="o")
            for bki in range(2):
                wt = wbig.tile([128, DCH, 512], BF, tag="wbig", name="wbig")
                nc.sync.dma_start(
                    out=wt[:],
                    in_=din[f'sa_wq_{i}'].ap().rearrange("(k p) n -> p k n", p=128)
                    [:, :, bki * 512:(bki + 1) * 512])
                for jj in range(4):
                    mt = bki * 4 + jj
                    q_ps = pst(128, T, "qproj")
                    for k in range(DCH):
                        nc.tensor.matmul(q_ps[:], lhsT=wt[:, k, 128 * jj:128 * (jj + 1)],
                                         rhs=xb[k][:], start=(k == 0), stop=(k == DCH - 1))
                    qraw = workA.tile([128, T], BF, tag="qraw", name="qraw")
                    nc.scalar.activation(qraw[:], q_ps[:], Copy)
                    qsw_ps = pst(128, T, "qsw", tag="m")
                    nc.tensor.matmul(qsw_ps[:], lhsT=pswap[:], rhs=qraw[:],
                                     start=True, stop=True)
                    sqq = workA.tile([128, T], F16, tag="sqq", name="sqq")
                    nc.scalar.activation(sqq[:], q_ps[:], Square)
                    nc.tensor.matmul(ssqq_ps[:], lhsT=bdall[:, mt, :],
                                     rhs=sqq[:], start=(mt == 0), stop=(mt == 7))
                    nc.vector.tensor_tensor(aq[mt][:], q_ps[:], rtq_c[:], MUL)
                    qsw_m = workA.tile([128, T], FP, tag="qswm", name="qswm")
                    nc.vector.tensor_tensor(qsw_m[:], qsw_ps[:], rtq_s[:], MUL)
                    nc.vector.tensor_tensor(aq[mt][:], aq[mt][:], qsw_m[:], ADD)
            qsc = batch_norm_scale(ssqq_ps, 16, rv2=rv2)
            for mt in range(DCH):
                rbch = pair_bcast(qsc, mt, rbc=rbc)
                nc.vector.tensor_tensor(qf[mt][:], aq[mt][:], rbch[:], MUL)

            # -- CA K (2 chunks over enc, width TK) --
            ksc_t = workB.tile([128, 2], FP, tag="ksc", name="ksc")
            nc.sync.dma_start(out=ksc_t[:],
                              in_=din[f'ca_kscale_{i}'].ap().rearrange("(t p) o -> p (t o)", p=128))
            wkt2 = wkv.tile([128, DCH, KVD], BF, tag="wkv", name="wkv")
            nc.sync.dma_start(out=wkt2[:],
                              in_=din[f'ca_wk_{i}'].ap().rearrange("(k p) n -> p k n", p=128))
            ssqc_ps = pst(128, TK, "ssqc", tag="o")
            ckraw = []
            for t in range(2):
                k_ps = pst(128, TK, "ckproj")
                for k in range(DCH):
                    nc.tensor.matmul(k_ps[:], lhsT=wkt2[:, k, 128 * t:128 * (t + 1)],
                                     rhs=encb[k][:], start=(k == 0), stop=(k == DCH - 1))
                kr = aqp.tile([128, TK], BF, tag=f"ckr{t}", name=f"ckr{t}")
                nc.scalar.activation(kr[:], k_ps[:], Copy)
                ckraw.append(kr)
                sqc = workA.tile([128, TK], F16, tag="sqc", name="sqc")
                nc.scalar.activation(sqc[:], k_ps[:], Square)
                nc.tensor.matmul(ssqc_ps[:], lhsT=bdall[:, t, :],
                                 rhs=sqc[:], start=(t == 0), stop=(t == 1))
            crr = batch_norm_scale(ssqc_ps, 4, width=TK)
            for t in range(2):
                rbch = pair_bcast(crr, t, nrows=4, width=TK)
                kh = workA.tile([128, TK], FP, tag="khca", name="khca")
                nc.vector.tensor_tensor(kh[:], ckraw[t][:], rbch[:], MUL)
                ckp = workB.tile([128, TK], BF, tag="ckp", name="ckp")
                nc.vector.tensor_scalar(
                    out=ckp[:], in0=kh[:],
                    scalar1=ksc_t[:, t:t + 1], scalar2=None, op0=MUL)
                for half in range(2):
                    kv = 2 * t + half
                    for dd in range(2):
                        nc.scalar.dma_start(out=kdca[kv][64 * dd:64 * (dd + 1), :],
                                            in_=ckp[64 * half:64 * (half + 1), :])

            # -- CA V (4 chunks of 128 enc tokens) --
            wvt2 = wkv.tile([128, DCH, KVD], BF, tag="wkv", name="wkv")
            nc.sync.dma_start(out=wvt2[:],
                              in_=din[f'ca_wv_{i}'].ap().rearrange("(k p) n -> p k n", p=128))
            for kv in range(KVH):
                nc.vector.memset(cv[kv][:, :, 64:65], 1.0)
                nc.vector.memset(cv[kv][:, :, 65:128], 0.0)
            for j in range(TP):
                v_ps = pst(128, KVD, "vproj", tag="m")
                for k in range(DCH):
                    nc.tensor.matmul(v_ps[:], lhsT=encb[k][:, 128 * j:128 * (j + 1)],
                                     rhs=wvt2[:, k, :], start=(k == 0), stop=(k == DCH - 1))
                for kv in range(KVH):
                    nc.scalar.activation(cv[kv][:, j, 0:64],
                                         v_ps[:, 64 * kv:64 * (kv + 1)], Copy)

            if DBG and i == 0:
                nc.sync.dma_start(out=dbg['tap_rbc'].ap(), in_=rbc[:])
                nc.sync.dma_start(out=dbg['tap_xb'].ap(), in_=xb[0][:])
                nc.sync.dma_start(out=dbg['tap_qf'].ap(), in_=qf[0][:])
            sa_attention(i)
            if DBG and i == 0:
                nc.sync.dma_start(out=dbg['tap_ao'].ap(), in_=ao[0][:])
            stream_out_proj(f'sa_wo_{i}')
            if DBG and i == 0:
                nc.sync.dma_start(out=dbg['tap_x1'].ap(), in_=x[0][:])

            # ---- LN2 (deferred) + CA ----
            rbc, rv2 = ln_prep()

            # -- CA Q (8 chunks, no rope) --
            ssqq_ps = pst(128, T, "ssqq", tag="o")
            qrawb = []
            for bki in range(2):
                wt = wbig.tile([128, DCH, 512], BF, tag="wbig", name="wbig")
                nc.sync.dma_start(
                    out=wt[:],
                    in_=din[f'ca_wq_{i}'].ap().rearrange("(k p) n -> p k n", p=128)
                    [:, :, bki * 512:(bki + 1) * 512])
                for jj in range(4):
                    mt = bki * 4 + jj
                    q_ps = pst(128, T, "qproj")
                    for k in range(DCH):
                        nc.tensor.matmul(q_ps[:], lhsT=wt[:, k, 128 * jj:128 * (jj + 1)],
                                         rhs=xb[k][:], start=(k == 0), stop=(k == DCH - 1))
                    qr = aqp.tile([128, T], BF, tag=f"cqr{mt}", name=f"cqr{mt}")
                    nc.scalar.activation(qr[:], q_ps[:], Copy)
                    qrawb.append(qr)
                    sqq = workA.tile([128, T], F16, tag="sqq", name="sqq")
                    nc.scalar.activation(sqq[:], q_ps[:], Square)
                    nc.tensor.matmul(ssqq_ps[:], lhsT=bdall[:, mt, :],
                                     rhs=sqq[:], start=(mt == 0), stop=(mt == 7))
            qsc = batch_norm_scale(ssqq_ps, 16, rv2=rv2)
            for mt in range(DCH):
                rbch = pair_bcast(qsc, mt, rbc=rbc)
                nc.vector.tensor_tensor(qf[mt][:], qrawb[mt][:], rbch[:], MUL)

            ca_attention(i)
            stream_out_proj(f'ca_wo_{i}')

            # ---- LN3 (deferred) + FFN ----
            rbc, rv2 = ln_prep()

            NF = F // 512
            for fb in range(NF):
                wgt = wbig.tile([128, DCH, 512], BF, tag="wbig", name="wbig")
                nc.sync.dma_start(
                    out=wgt[:],
                    in_=din[f'ffn_wg_{i}'].ap().rearrange("(k p) n -> p k n", p=128)
                    [:, :, fb * 512:(fb + 1) * 512])
                wut = wbig.tile([128, DCH, 512], BF, tag="wbig", name="wbig")
                nc.sync.dma_start(
                    out=wut[:],
                    in_=din[f'ffn_wu_{i}'].ap().rearrange("(k p) n -> p k n", p=128)
                    [:, :, fb * 512:(fb + 1) * 512])
                wdt = wbig.tile([128, 4, D], BF, tag="wbig", name="wbig")
                nc.sync.dma_start(
                    out=wdt[:],
                    in_=din[f'ffn_wd_{i}'].ap().rearrange("(k p) n -> p k n", p=128)
                    [:, fb * 4:(fb + 1) * 4, :])
                prods = []
                for hf in range(2):
                    gu = []
                    for which, wt in (('g', wgt), ('u', wut)):
                        g_ps = pst(128, 512, "s")
                        for jj in range(2):
                            j = 2 * hf + jj
                            for k in range(DCH):
                                nc.tensor.matmul(
                                    g_ps[:, 256 * jj:256 * (jj + 1)],
                                    lhsT=wt[:, k, 128 * j:128 * (j + 1)],
                                    rhs=xb[k][:], start=(k == 0), stop=(k == DCH - 1))
                        g_sb = ffnp.tile([128, 512], BF, tag=f"relu{which}", name=f"relu{which}")
                        if which == 'g':
                            nc.scalar.activation(g_sb[:], g_ps[:], Relu)
                        else:
                            nc.vector.tensor_scalar(out=g_sb[:], in0=g_ps[:],
                                                    scalar1=0.0, scalar2=None,
                                                    op0=AluOpType.max)
                        gu.append(g_sb)
                    pr = ffnp.tile([128, 512], BF, tag="prod", name="prod", bufs=3)
                    nc.vector.tensor_tensor(pr[:], gu[0][:], gu[1][:], MUL)
                    prods.append(pr)
                for m in range(DCH):
                    yp = pst(128, T, "yp", tag="m")
                    for kc in range(4):
                        nc.tensor.matmul(
                            yp[:],
                            lhsT=wdt[:, kc, 128 * m:128 * (m + 1)],
                            rhs=prods[kc // 2][:, 256 * (kc % 2):256 * (kc % 2 + 1)],
                            start=(kc == 0), stop=(kc == 3))
                    if fb == 0:
                        nc.scalar.activation(y_sb[m][:], yp[:], Copy)
                    else:
                        nc.vector.tensor_tensor(y_sb[m][:], y_sb[m][:], yp[:], ADD)
            for m in range(DCH):
                nc.vector.tensor_tensor(y_sb[m][:], y_sb[m][:], rv2[:], MUL)
                nc.vector.tensor_tensor(x[m][:], x[m][:], y_sb[m][:], ADD)

        # ---- final norm + output ----
        ssf = pst(128, T, "ssf", tag="o")
        for m in range(DCH):
            sq = workA.tile([128, T], F16, tag="sq", name="sq")
            nc.vector.tensor_tensor(sq[:], x[m][:], x[m][:], MUL)
            nc.tensor.matmul(ssf[:], lhsT=ones[:], rhs=sq[:],
                             start=(m == 0), stop=(m == DCH - 1))
        lnf = workB.tile([128, T], FP, tag="lnf", name="lnf")
        nc.scalar.activation(lnf[:], ssf[:], Ln, bias=eps_t[:], scale=1.0 / D)
        rbcf = workA.tile([128, T], FP, tag="rbcf", name="rbcf")
        nc.scalar.activation(rbcf[:], lnf[:], Exp, scale=-0.5)
        for m in range(DCH):
            ot = workB.tile([128, T], FP, tag="otile", name="otile", bufs=2)
            nc.vector.tensor_tensor(ot[:], x[m][:], rbcf[:], MUL)
            nc.vector.tensor_scalar(out=ot[:], in0=ot[:],
                                    scalar1=fscale[:, m:m + 1], scalar2=None, op0=MUL)
            nc.sync.dma_start(out=out_dram.ap()[128 * m:128 * (m + 1), :], in_=ot[:])

    nc.compile()
    return nc


def _get_program():
    global _PROG
    if _PROG is None:
        _PROG = _build_program()
    return _PROG


def kernel(**inputs):
    from concourse import bass_utils
    host, per_core = host_prepare(inputs)
    nc = _get_program()
    in_maps = []
    for c in range(NCORES):
        m = dict(per_core[c])
        for k, v in host.items():
            if k.startswith('rope'):
                continue  # per-core sliced versions already present
            m[k] = v
        in_maps.append(m)
    res = bass_utils.run_bass_kernel_spmd(nc, in_maps, list(range(NCORES)))
    out = np.empty((B, TQ, D), np.float32)
    for c in range(NCORES):
        grp, r = c // TP, c % TP
        out[grp, r::TP] = res.results[c]['outT'].T
    return out



# revision 50
# speedup vs baseline: 1.1193x; 1.0263x over previous
"""Self-contained Trainium2 Bass kernel for nn_EncoderDecoderTransformer_90941637525663.

Sequence-parallel over 8 NeuronCores (2 batch groups x 4 token shards), with
INTERLEAVED token assignment (core r owns tokens t = 4j + r): causal
visibility becomes structurally uniform across cores, so each head-pair
processes 12 score blocks instead of 20 (full blocks need no mask; the four
diagonal blocks use a per-core triangular mask loaded as input). Activations
live transposed in SBUF (feature dim on partitions, tokens on free dim).
Per layer one AllGather exchanges K (normalized + roped) and V (pre-scaled
by the sender 1/rms row), with Q projection and cross-attention K/V filling
the gather latency. All heavy matmuls run in bf16; helper selector and
accumulator weights (ones/bdall/sel16/squares/scales) are fp16 so they stay
on the fast full-width weight path. All rms factors use exp(-0.5*ln(mean+eps))
so every activation lives in one ACT table set (a single table load total,
vs ~1.3us per sqrt<->exp switch). Softmax row-sum division is batched per
head pair: vector reciprocal + gpsimd partition-broadcast. LN scales are
folded into consumer weights host-side; deferred-LN per-head rms folds rinv
in via rv2.
"""
import sys
sys.path.insert(0, '/opt/trn_rl_repo')
import numpy as np

B, TQ, TK, D, H, KVH, L, F = 2, 1024, 512, 1024, 16, 4, 2, 4096
HD, KVD = 64, 256
EPS = 1e-6
NCORES, TP = 8, 4
T = TQ // TP           # 256 tokens per core (interleaved: t = 4j + r)
DCH = D // 128         # 8 feature chunks
NEG = -3.0e38
AGR = 2 * T            # ag rows: [0,T)=k (normalized), [T,2T)=v (rinv-scaled)


def _bf16():
    import ml_dtypes
    return ml_dtypes.bfloat16


def _rope_tables(Tlen, hd, theta=10000.0):
    freqs = 1.0 / theta ** (np.arange(0, hd, 2, dtype=np.float32) / hd)
    ang = np.outer(np.arange(Tlen, dtype=np.float32), freqs)
    return np.cos(ang).astype(np.float32), np.sin(ang).astype(np.float32)


def host_prepare(inputs):
    """Returns (host, per_core): folded shared arrays + per-core arrays."""
    bf16 = _bf16()
    inputs = {k: np.ascontiguousarray(np.asarray(v, dtype=np.float32))
              for k, v in inputs.items()}
    cos_f, sin_f = _rope_tables(TQ, HD)       # [TQ, 32]

    def bf(a):
        return np.ascontiguousarray(a.astype(bf16))

    fp16 = np.float16

    host = {}
    for i in range(L):
        ln1 = (1.0 + inputs['ln1_s'][i])[:, None]
        ln2 = (1.0 + inputs['ln2_s'][i])[:, None]
        ln3 = (1.0 + inputs['ln3_s'][i])[:, None]
        host[f'sa_wq_{i}'] = bf(ln1 * inputs['sa_wq'][i])
        host[f'sa_wk_{i}'] = bf(ln1 * inputs['sa_wk'][i])
        host[f'sa_wv_{i}'] = bf(ln1 * inputs['sa_wv'][i])
        host[f'sa_wo_{i}'] = bf(inputs['sa_wo'][i])
        host[f'ca_wq_{i}'] = bf(ln2 * inputs['ca_wq'][i])
        host[f'ca_wk_{i}'] = bf(inputs['ca_wk'][i])
        host[f'ca_wv_{i}'] = bf(inputs['ca_wv'][i])
        host[f'ca_wo_{i}'] = bf(inputs['ca_wo'][i])
        host[f'ffn_wg_{i}'] = bf(ln3 * inputs['ffn_wg'][i])
        host[f'ffn_wu_{i}'] = bf(ln3 * inputs['ffn_wu'][i])
        host[f'ffn_wd_{i}'] = bf(inputs['ffn_wd'][i])
        # rope tables with (1+qn)/(1+kn) head-dim scales folded; [128, TQ]
        for which, dvec in [('q', inputs['sa_qn'][i]), ('k', inputs['sa_kn'][i])]:
            d1, d2 = 1.0 + dvec[:32], 1.0 + dvec[32:]
            C = np.concatenate([d1[:, None] * cos_f.T, d2[:, None] * cos_f.T], 0)
            S = np.concatenate([-d2[:, None] * sin_f.T, d1[:, None] * sin_f.T], 0)
            host[f'rope{which}_c_{i}'] = np.ascontiguousarray(np.concatenate([C, C], 0))
            host[f'rope{which}_s_{i}'] = np.ascontiguousarray(np.concatenate([S, S], 0))
        sc = ((1.0 + inputs['ca_qn'][i]) * (1.0 + inputs['ca_kn'][i])).astype(np.float32)
        host[f'ca_kscale_{i}'] = np.tile(sc, KVH)[:, None].copy()   # [256, 1]
    host['final_scale'] = (1.0 + inputs['final_s'])[:, None].copy()  # [D, 1]
    # bdall[:, mt, :]: [128,128] stationary (fp16, M padded to 128 for fast
    # weight load) accumulating chunk mt's per-head-pair row sums into rows
    # 2mt/2mt+1 of a [128,T] psum (other rows zero)
    bdall = np.zeros((128, 8, 128), fp16)
    for mt in range(8):
        bdall[0:64, mt, 2 * mt] = 1.0
        bdall[64:128, mt, 2 * mt + 1] = 1.0
    host['bdall'] = bdall
    # sel16[0:n, mt, :]: [n,128] stationary (fp16) broadcasting rows 2mt/2mt+1
    # of an [n,T] scale tile to the 64-row head halves of a [128,T] output
    sel16 = np.zeros((16, 8, 128), fp16)
    for mt in range(8):
        sel16[2 * mt, mt, 0:64] = 1.0
        sel16[2 * mt + 1, mt, 64:128] = 1.0
    host['sel16'] = sel16
    # rope half-swap permutation (within each 64-row head block): [128, 128]
    psw = np.zeros((128, 128), np.float32)
    for base in (0, 64):
        for r in range(32):
            psw[base + 32 + r, base + r] = 1.0      # out row base+r <- in row base+32+r
            psw[base + r, base + 32 + r] = 1.0      # out row base+32+r <- in row base+r
    host['pswap'] = bf(psw)

    per_core = []
    kl = np.arange(128)[:, None]
    ql = np.arange(128)[None, :]
    tri_inc = np.where(kl <= ql, 0.0, NEG).astype(np.float32)
    tri_exc = np.where(kl < ql, 0.0, NEG).astype(np.float32)
    for c in range(NCORES):
        grp, r = c // TP, c % TP
        tok = slice(r, None, TP)   # interleaved tokens: 4j + r
        pc = {
            'xT': np.ascontiguousarray(inputs['x'][grp].T[:, tok]),
            'encTb': bf(inputs['encoder_out'][grp].T),
        }
        # diagonal-chunk additive mask vs source core g: k-token 4p+g visible
        # to q-token 4j+r within the same 128-block iff p<=j (g<=r) / p<j (g>r)
        dm = np.empty((128, TP, 128), np.float32)
        for g in range(TP):
            dm[:, g, :] = tri_inc if g <= r else tri_exc
        pc['dmask'] = np.ascontiguousarray(dm)
        for i in range(L):
            for nm in ('ropeq_c', 'ropeq_s', 'ropek_c', 'ropek_s'):
                pc[f'{nm}_{i}'] = np.ascontiguousarray(host[f'{nm}_{i}'][:, tok])
        per_core.append(pc)
    return host, per_core


_PROG = None
DBG = False


def _build_program(nlayers=L):
    import concourse.bass as bass
    import concourse.tile as tile
    from concourse import bacc, mybir
    from concourse.alu_op_type import AluOpType
    from contextlib import ExitStack

    R = mybir.dt.float32r
    FP = mybir.dt.float32
    BF = mybir.dt.bfloat16
    F16 = mybir.dt.float16
    Exp = mybir.ActivationFunctionType.Exp
    Ln = mybir.ActivationFunctionType.Ln
    Relu = mybir.ActivationFunctionType.Relu
    Square = mybir.ActivationFunctionType.Square
    Copy = mybir.ActivationFunctionType.Copy

    class _Bacc(bacc.Bacc):
        def insert_act_table_loads(self):
            # Force exp/ln (this kernel's only table-bound activations) to
            # resolve to the combined natural_log_exp_and_others set: the
            # default first-match selection alternates exp_and_others /
            # natural_log and pays a ~1.3us table load per switch.
            import bass_rust as _bass_rust
            has_activation = any(
                isinstance(ins, mybir.InstActivation)
                for b in self.main_func.blocks
                for ins in b.instructions
            )
            if not has_activation:
                return
            from concourse.hw_specs import get_activation_tables
            tables = list(get_activation_tables(self.m.arch).items())
            combined = next(i for i, (nm, _) in enumerate(tables)
                            if nm == 'natural_log_exp_and_others')
            tables = [(nm, s if i == combined else set())
                      for i, (nm, s) in enumerate(tables)]
            _bass_rust.insert_act_table_loads(self, tables)

    nc = _Bacc("TRN2", target_bir_lowering=False, debug=False,
               num_devices=NCORES)

    din = {}
    def dri(name, shape, dt):
        din[name] = nc.dram_tensor(name, list(shape), dt, kind="ExternalInput")

    dri('xT', (D, T), FP)
    dri('encTb', (D, TK), BF)
    dri('dmask', (128, TP, 128), FP)
    dri('final_scale', (D, 1), FP)
    dri('bdall', (128, 8, 128), F16)
    dri('sel16', (16, 8, 128), F16)
    dri('pswap', (128, 128), BF)
    for i in range(nlayers):
        dri(f'sa_wq_{i}', (D, D), BF); dri(f'sa_wk_{i}', (D, KVD), BF)
        dri(f'sa_wv_{i}', (D, KVD), BF); dri(f'sa_wo_{i}', (D, D), BF)
        dri(f'ca_wq_{i}', (D, D), BF); dri(f'ca_wk_{i}', (D, KVD), BF)
        dri(f'ca_wv_{i}', (D, KVD), BF); dri(f'ca_wo_{i}', (D, D), BF)
        dri(f'ffn_wg_{i}', (D, F), BF); dri(f'ffn_wu_{i}', (D, F), BF)
        dri(f'ffn_wd_{i}', (F, D), BF)
        for nm in ('ropeq_c', 'ropeq_s', 'ropek_c', 'ropek_s'):
            dri(f'{nm}_{i}', (128, T), FP)
        dri(f'ca_kscale_{i}', (KVD, 1), FP)
    out_dram = nc.dram_tensor('outT', [D, T], FP, kind="ExternalOutput")
    dbg = {}
    if DBG:
        for nm, shp, dt in [('tap_rbc', [128, T], FP), ('tap_xb', [128, T], BF),
                            ('tap_qsc', [16, T], FP), ('tap_qf', [128, T], BF),
                            ('tap_kd', [128, T], BF), ('tap_vown', [128, 256], BF),
                            ('tap_kfull', [128, TQ], BF), ('tap_vfull', [128, TP * 2 * 128], BF),
                            ('tap_rvg', [128, TP * 2], BF), ('tap_ao', [128, T], BF),
                            ('tap_x1', [128, T], FP)]:
            dbg[nm] = nc.dram_tensor(nm, shp, dt, kind="ExternalOutput")
    ag_in = [nc.dram_tensor(f'ag_in_{i}', [AGR, T], BF) for i in range(nlayers)]
    ag_out = [nc.dram_tensor(f'ag_out_{i}', [AGR * TP, T], BF) for i in range(nlayers)]
    agw_in = nc.dram_tensor('agw_in', [1, 128], BF)
    agw_out = nc.dram_tensor('agw_out', [TP, 128], BF)
    GROUPS = [[0, 1, 2, 3], [4, 5, 6, 7]]

    with nc.allow_low_precision(reason="bf16 pipeline"), \
            tile.TileContext(nc) as tc, ExitStack() as ctx:
        consts = ctx.enter_context(tc.tile_pool(name="consts", bufs=1))
        state = ctx.enter_context(tc.tile_pool(name="state", bufs=1))
        aqp = ctx.enter_context(tc.tile_pool(name="aqp", bufs=1))
        kvf = ctx.enter_context(tc.tile_pool(name="kvf", bufs=2))
        wbig = ctx.enter_context(tc.tile_pool(name="wbig", bufs=3))
        wkv = ctx.enter_context(tc.tile_pool(name="wkv", bufs=2))
        workA = ctx.enter_context(tc.tile_pool(name="workA", bufs=2))
        workB = ctx.enter_context(tc.tile_pool(name="workB", bufs=2))
        psb = ctx.enter_context(tc.tile_pool(name="psb", bufs=4))
        ffnp = ctx.enter_context(tc.tile_pool(name="ffnp", bufs=2))
        ropep = ctx.enter_context(tc.tile_pool(name="ropep", bufs=2))
        ps = ctx.enter_context(tc.tile_pool(name="ps", bufs=8, space="PSUM"))

        def pst(p_, f_, name, tag="s"):
            bufs = {"s": 3, "o": 2, "m": 3}[tag]
            return ps.tile([p_, f_], FP, tag=f"ps_{tag}", name=name, bufs=bufs)

        MUL, ADD = AluOpType.mult, AluOpType.add

        # ---- warm-up collective: the first collective's entry barrier
        # absorbs inter-core launch skew; run it on a tiny buffer up front so
        # the wait overlaps constant/weight loads and the pre-AG compute ----
        warm = consts.tile([1, 128], BF, tag="warm", name="warm")
        nc.vector.memset(warm[:], 0.0)
        nc.sync.dma_start(out=agw_in.ap(), in_=warm[:])
        nc.gpsimd.collective_compute(
            "AllGather", mybir.AluOpType.bypass, replica_groups=GROUPS,
            ins=[agw_in.ap().opt()], outs=[agw_out.ap().opt()])

        # ---- constants ----
        ones = consts.tile([128, 128], F16, tag="ones", name="ones")
        nc.vector.memset(ones[:], 1.0)
        bdall = consts.tile([128, 8, 128], F16, tag="bdall", name="bdall")
        nc.sync.dma_start(out=bdall[:], in_=din['bdall'].ap())
        sel16 = consts.tile([16, 8, 128], F16, tag="sel16", name="sel16")
        nc.sync.dma_start(out=sel16[:], in_=din['sel16'].ap())
        eps_t = consts.tile([128, 1], FP, tag="eps", name="eps")
        nc.vector.memset(eps_t[:], EPS)
        id1 = consts.tile([1, 1], FP, tag="id1", name="id1")
        nc.vector.memset(id1[:], 1.0)
        pswap = consts.tile([128, 128], BF, tag="pswap", name="pswap")
        nc.sync.dma_start(out=pswap[:], in_=din['pswap'].ap())
        dmask = consts.tile([128, TP, 128], FP, tag="dmask", name="dmask")
        nc.sync.dma_start(out=dmask[:], in_=din['dmask'].ap())
        fscale = consts.tile([128, DCH], FP, tag="fscale", name="fscale")
        nc.sync.dma_start(out=fscale[:],
                          in_=din['final_scale'].ap().rearrange("(k p) o -> p (k o)", p=128))

        # ---- persistent state ----
        x = [state.tile([128, T], FP, tag=f"x{m}", name=f"x{m}") for m in range(DCH)]
        for m in range(DCH):
            nc.sync.dma_start(out=x[m][:], in_=din['xT'].ap()[128 * m:128 * (m + 1), :])
        xb = [state.tile([128, T], BF, tag=f"xb{m}", name=f"xb{m}") for m in range(DCH)]
        encb = [state.tile([128, TK], BF, tag=f"enc{m}", name=f"enc{m}") for m in range(DCH)]
        for m in range(DCH):
            nc.sync.dma_start(out=encb[m][:], in_=din['encTb'].ap()[128 * m:128 * (m + 1), :])
        ao = [state.tile([128, T], BF, tag=f"ao{m}", name=f"ao{m}") for m in range(DCH)]
        qf = [state.tile([128, T], BF, tag=f"qf{t}", name=f"qf{t}") for t in range(H // 2)]
        kdca = [state.tile([128, TK], BF, tag=f"kdca{k}", name=f"kdca{k}") for k in range(KVH)]
        cv = [state.tile([128, TP, 128], BF, tag=f"cv{k}", name=f"cv{k}") for k in range(KVH)]
        y_sb = [state.tile([128, T], FP, tag=f"ysb{m}", name=f"ysb{m}") for m in range(DCH)]
        aq = y_sb

        # ---------------- helpers ----------------
        def ln_prep(tagp=""):
            """rms over D partitions of fp32 x -> (rbc [128,T] FP broadcast of
            1/rms, rv2 [128,T] FP = rbc^2). Also refreshes xb (bf16 cast of x).
            Uses exp(-0.5*ln(mean+eps)) on full 128 rows so no partition
            broadcast is needed (ACT cost is free-dim bound)."""
            ss = pst(128, T, "ss", tag="o")
            for m in range(DCH):
                sq = workA.tile([128, T], F16, tag="sq", name="sq")
                nc.vector.tensor_tensor(sq[:], x[m][:], x[m][:], MUL)
                nc.tensor.matmul(ss[:], lhsT=ones[:], rhs=sq[:],
                                 start=(m == 0), stop=(m == DCH - 1))
                nc.vector.tensor_copy(xb[m][:], x[m][:])
            lnv = workA.tile([128, T], FP, tag="lnv" + tagp, name="lnv")
            nc.scalar.activation(lnv[:], ss[:], Ln, bias=eps_t[:], scale=1.0 / D)
            rbc = workA.tile([128, T], FP, tag="rbc" + tagp, name="rbc")
            nc.scalar.activation(rbc[:], lnv[:], Exp, scale=-0.5)
            rv2 = workA.tile([128, T], FP, tag="rv2" + tagp, name="rv2")
            nc.vector.tensor_tensor(rv2[:], rbc[:], rbc[:], MUL)
            return rbc, rv2

        def batch_norm_scale(ssq_ps, nrows, rv2=None, width=T):
            """Per-head-pair raw sums [nrows, width] psum -> 1/rms scale (F16)
            via exp(-0.5*ln(mean+eps)). With rv2: deferred-LN variant (sums
            scaled by rinv^2 first)."""
            if rv2 is not None:
                tk = workA.tile([nrows, width], FP, tag="bnt", name="bnt",
                                padded_shape=[16, TK])
                nc.vector.tensor_tensor(tk[:], ssq_ps[0:nrows, 0:width],
                                        rv2[0:nrows, :width], MUL)
                src = tk[:]
            else:
                src = ssq_ps[0:nrows, 0:width]
            sr = workB.tile([nrows, width], FP, tag="bns", name="bns",
                            padded_shape=[16, TK])
            nc.scalar.activation(sr[:], src, Ln, bias=eps_t[0:nrows, :], scale=1.0 / HD)
            rr = workB.tile([nrows, width], F16, tag="bnr", name="bnr",
                            padded_shape=[16, TK])
            nc.scalar.activation(rr[:], sr[:], Exp, scale=-0.5)
            return rr

        def pair_bcast(scale_tile, t, nrows=16, width=T, rbc=None):
            """Broadcast rows 2t/2t+1 of an [nrows, width] F16 scale tile to the
            64-row halves of a [128, width] FP tile via PE selector matmul.
            If rbc given, fold the per-token rinv into the drain."""
            bc_ps = pst(128, width, "bch", tag="m")
            nc.tensor.matmul(bc_ps[:], lhsT=sel16[0:nrows, t, :],
                             rhs=scale_tile[:],
                             start=True, stop=True)
            rbch = workA.tile([128, width], FP, tag="rbch", name="rbch",
                              padded_shape=[128, TK])
            if rbc is not None:
                nc.vector.tensor_tensor(rbch[:], bc_ps[:], rbc[:, :width], MUL)
            else:
                nc.scalar.activation(rbch[:], bc_ps[:], Copy)
            return rbch

        def norm_pair(qt, o2):
            """Divide both heads' outputs by their softmax rowsums (row 64 of
            each o2): recip on vector, partition-broadcast on the otherwise
            idle gpsimd, final muls on vector."""
            rs2 = workB.tile([1, 2, T], FP, tag="rs2", name="rs2", bufs=3)
            for h in range(2):
                nc.scalar.activation(rs2[0:1, h, :], o2[h][64:65, :], Copy)
            rr2 = workB.tile([1, 2, T], FP, tag="rr2", name="rr2", bufs=3)
            nc.vector.reciprocal_approx_fast(rr2[:], rs2[:])
            bc_sb = workB.tile([128, 2, T], FP, tag="bc_sb", name="bc_sb", bufs=3)
            nc.gpsimd.partition_broadcast(bc_sb[:], rr2[:])
            for h, par in ((0, 0), (1, 64)):
                nc.vector.tensor_tensor(ao[qt][par:par + 64, :],
                                        o2[h][0:64, :],
                                        bc_sb[par:par + 64, h, :], MUL)

        def sa_attention(i):
            ag = ag_out[i].ap()
            for kv in range(KVH):
                kga = kvf.tile([128, TP, T], BF, tag="kfull", name="kfull")
                src = bass.AP(tensor=ag.tensor, offset=(64 * kv) * T,
                              ap=[[T, 64], [AGR * T, TP], [1, T]])
                for dd in range(2):
                    nc.sync.dma_start(out=kga[64 * dd:64 * (dd + 1), :, :], in_=src)
                vga = kvf.tile([128, TP, 2, 128], BF, tag="vfull", name="vfull")
                nc.vector.memset(vga[:, :, :, 64:65], 1.0)
                nc.vector.memset(vga[:, :, :, 65:128], 0.0)
                for cb in range(2):
                    vap = bass.AP(tensor=ag.tensor,
                                  offset=T * T + 128 * T * cb + 64 * kv,
                                  ap=[[T, 128], [AGR * T, TP], [1, 64]])
                    nc.sync.dma_start(out=vga[:, :, cb, 0:64], in_=vap)
                for pp in range(2):
                    qt = 2 * kv + pp
                    o2 = [pst(128, T, f"o{h}", tag="o") for h in range(2)]
                    mi = 0
                    # C=0 chunks [128k x 256q]: left half (vs J0) is the
                    # diagonal (masked), right half (vs J1) fully visible
                    for b in range(2):
                        s2t = [pst(128, 512, "s") for _ in range(2)]
                        for gg in range(2):
                            g = 2 * b + gg
                            for h, par in ((0, 0), (1, 64)):
                                nc.tensor.matmul(
                                    s2t[h][:, 256 * gg:256 * (gg + 1)],
                                    lhsT=kga[par:par + 64, g, 0:128],
                                    rhs=qf[qt][par:par + 64, :],
                                    start=True, stop=True)
                        p2t = []
                        for h in range(2):
                            sv = s2t[h][:].rearrange("p (c t) -> p c t", c=2)
                            nc.vector.tensor_tensor(
                                sv[:, :, 0:128], sv[:, :, 0:128],
                                dmask[:, 2 * b:2 * b + 2, :], ADD)
                            p_sb = psb.tile([128, 2, T], BF, tag="p_sb", name="p_sb")
                            nc.scalar.activation(p_sb[:], sv, Exp, scale=0.125)
                            p2t.append(p_sb)
                        for gg in range(2):
                            g = 2 * b + gg
                            for h in range(2):
                                nc.tensor.matmul(o2[h][:], lhsT=vga[:, g, 0, :],
                                                 rhs=p2t[h][:, gg, :],
                                                 start=(mi == 0), stop=False,
                                                 skip_group_check=True)
                            mi += 1
                    # C=1 chunks [128k x 128q]: all diagonal, right half only
                    s2t = [pst(128, 512, "s") for _ in range(2)]
                    for h, par in ((0, 0), (1, 64)):
                        for g in range(TP):
                            nc.tensor.matmul(
                                s2t[h][:, 128 * g:128 * (g + 1)],
                                lhsT=kga[par:par + 64, g, 128:256],
                                rhs=qf[qt][par:par + 64, 128:256],
                                start=True, stop=True)
                    p2t = []
                    for h in range(2):
                        sv = s2t[h][:].rearrange("p (g t) -> p g t", g=TP)
                        nc.vector.tensor_tensor(sv, sv, dmask[:], ADD)
                        p_sb = psb.tile([128, TP, 128], BF, tag="p_sb", name="p_sb")
                        nc.scalar.activation(p_sb[:], sv, Exp, scale=0.125)
                        p2t.append(p_sb)
                    for g in range(TP):
                        for h in range(2):
                            nc.tensor.matmul(o2[h][:, 128:256], lhsT=vga[:, g, 1, :],
                                             rhs=p2t[h][:, g, :],
                                             start=False, stop=(g == TP - 1),
                                             skip_group_check=True)
                    norm_pair(qt, o2)

        def ca_attention(i):
            nk_chunks = TK // 128
            for kv in range(KVH):
                ksrc = kdca[kv]
                vsrc = cv[kv]
                for pp in range(2):
                    # process both heads of pair tile qt together: their score
                    # matmuls use disjoint PE row groups (lhsT base 0 vs 64) and
                    # overlap when issued back-to-back
                    qt = 2 * kv + pp
                    o2 = [pst(128, T, f"o{h}", tag="o") for h in range(2)]
                    mi = 0
                    for half in range(nk_chunks // 2):
                        s2t = [pst(128, 512, "s") for _ in range(2)]
                        for cc in range(2):
                            c = 2 * half + cc
                            for h, par in ((0, 0), (1, 64)):
                                nc.tensor.matmul(
                                    s2t[h][:, 256 * cc:256 * (cc + 1)],
                                    lhsT=ksrc[par:par + 64, 128 * c:128 * (c + 1)],
                                    rhs=qf[qt][par:par + 64, :], start=True, stop=True)
                        p2t = []
                        for h in range(2):
                            p_sb = psb.tile([128, 2, T], BF, tag="p_sb", name="p_sb")
                            nc.scalar.activation(p_sb[:], s2t[h][:].rearrange(
                                "p (c t) -> p c t", c=2), Exp, scale=0.125)
                            p2t.append(p_sb)
                        for cc in range(2):
                            c = 2 * half + cc
                            for h in range(2):
                                nc.tensor.matmul(o2[h][:], lhsT=vsrc[:, c, :],
                                                 rhs=p2t[h][:, cc, :],
                                                 start=(mi == 0),
                                                 stop=(mi == nk_chunks - 1),
                                                 skip_group_check=True)
                            mi += 1
                    norm_pair(qt, o2)

        def stream_out_proj(w_name):
            for bki in range(2):
                wt = wbig.tile([128, DCH, 512], BF, tag="wbig", name="wbig")
                nc.sync.dma_start(
                    out=wt[:],
                    in_=din[w_name].ap().rearrange("(k p) n -> p k n", p=128)
                    [:, :, bki * 512:(bki + 1) * 512])
                for j in range(4):
                    m = bki * 4 + j
                    y_ps = pst(128, T, "proj", tag="m")
                    for k in range(DCH):
                        nc.tensor.matmul(y_ps[:], lhsT=wt[:, k, 128 * j:128 * (j + 1)],
                                         rhs=ao[k][:],
                                         start=(k == 0), stop=(k == DCH - 1))
                    nc.vector.tensor_tensor(x[m][:], x[m][:], y_ps[:], ADD)

        # ================= layers =================
        for i in range(nlayers):
            # ---- LN1 (deferred) + SA ----
            rbc, rv2 = ln_prep()
            rvT = []
            for j in range(2):
                tp_ps = pst(128, 1, "tp", tag="m")
                nc.tensor.transpose(tp_ps[:], rbc[0:1, 128 * j:128 * (j + 1)], id1[:])
                rv_b = workB.tile([128, 1], FP, tag=f"rvT{j}", name=f"rvT{j}")
                nc.scalar.activation(rv_b[:], tp_ps[:], Copy)
                rvT.append(rv_b)

            rtq_c = ropep.tile([128, T], FP, tag="rtc", name="rtc")
            nc.sync.dma_start(out=rtq_c[:], in_=din[f'ropeq_c_{i}'].ap())
            rtq_s = ropep.tile([128, T], FP, tag="rts", name="rts")
            nc.sync.dma_start(out=rtq_s[:], in_=din[f'ropeq_s_{i}'].ap())
            rtk_c = ropep.tile([128, T], FP, tag="rtc", name="rtc")
            nc.sync.dma_start(out=rtk_c[:], in_=din[f'ropek_c_{i}'].ap())
            rtk_s = ropep.tile([128, T], FP, tag="rts", name="rts")
            nc.sync.dma_start(out=rtk_s[:], in_=din[f'ropek_s_{i}'].ap())

            # -- SA K (2 chunks of 128 cols) --
            wkt = wkv.tile([128, DCH, KVD], BF, tag="wkv", name="wkv")
            nc.sync.dma_start(out=wkt[:],
                              in_=din[f'sa_wk_{i}'].ap().rearrange("(k p) n -> p k n", p=128))
            ssqk_ps = pst(128, T, "ssqk", tag="o")
            ka = []
            for t in range(2):
                k_ps = pst(128, T, "kproj")
                for k in range(DCH):
                    nc.tensor.matmul(k_ps[:], lhsT=wkt[:, k, 128 * t:128 * (t + 1)],
                                     rhs=xb[k][:], start=(k == 0), stop=(k == DCH - 1))
                kraw = workA.tile([128, T], BF, tag="kraw", name="kraw")
                nc.scalar.activation(kraw[:], k_ps[:], Copy)
                ksw_ps = pst(128, T, "ksw", tag="m")
                nc.tensor.matmul(ksw_ps[:], lhsT=pswap[:], rhs=kraw[:],
                                 start=True, stop=True)
                sqk = workA.tile([128, T], F16, tag="sqk", name="sqk")
                nc.scalar.activation(sqk[:], k_ps[:], Square)
                nc.tensor.matmul(ssqk_ps[:], lhsT=bdall[:, t, :],
                                 rhs=sqk[:], start=(t == 0), stop=(t == 1))
                kat = workB.tile([128, T], FP, tag=f"ka{t}", name=f"ka{t}")
                nc.vector.tensor_tensor(kat[:], k_ps[:], rtk_c[:], MUL)
                ksw_m = workA.tile([128, T], FP, tag="kswm", name="kswm")
                nc.vector.tensor_tensor(ksw_m[:], ksw_ps[:], rtk_s[:], MUL)
                nc.vector.tensor_tensor(kat[:], kat[:], ksw_m[:], ADD)
                ka.append(kat)
            ksc = batch_norm_scale(ssqk_ps, 4, rv2=rv2)
            for t in range(2):
                rbch = pair_bcast(ksc, t, nrows=4, rbc=rbc)
                kf = workA.tile([128, T], BF, tag="kf", name="kf")
                nc.vector.tensor_tensor(kf[:], ka[t][:], rbch[:], MUL)
                nc.sync.dma_start(out=ag_in[i].ap()[128 * t:128 * (t + 1), :],
                                  in_=kf[:])

            # -- SA V (2 token chunks of 128), rinv-scaled before the gather --
            wvt = wkv.tile([128, DCH, KVD], BF, tag="wkv", name="wkv")
            nc.sync.dma_start(out=wvt[:],
                              in_=din[f'sa_wv_{i}'].ap().rearrange("(k p) n -> p k n", p=128))
            for j in range(2):
                v_ps = pst(128, KVD, "vproj", tag="m")
                for k in range(DCH):
                    nc.tensor.matmul(v_ps[:], lhsT=xb[k][:, 128 * j:128 * (j + 1)],
                                     rhs=wvt[:, k, :], start=(k == 0), stop=(k == DCH - 1))
                vraw = workA.tile([128, KVD], BF, tag="vraw", name="vraw")
                nc.vector.tensor_scalar(out=vraw[:], in0=v_ps[:],
                                        scalar1=rvT[j][:], scalar2=None, op0=MUL)
                nc.sync.dma_start(out=ag_in[i].ap()[T + 128 * j:T + 128 * (j + 1), :],
                                  in_=vraw[:])

            nc.gpsimd.collective_compute(
                "AllGather", mybir.AluOpType.bypass, replica_groups=GROUPS,
                ins=[ag_in[i].ap().opt()], outs=[ag_out[i].ap().opt()])

            # -- SA Q (8 chunks) --
            ssqq_ps = pst(128, T, "ssqq", tag="o")
            for bki in range(2):
                wt = wbig.tile([128, DCH, 512], BF, tag="wbig", name="wbig")
                nc.sync.dma_start(
                    out=wt[:],
                    in_=din[f'sa_wq_{i}'].ap().rearrange("(k p) n -> p k n", p=128)
                    [:, :, bki * 512:(bki + 1) * 512])
                for jj in range(4):
                    mt = bki * 4 + jj
                    q_ps = pst(128, T, "qproj")
                    for k in range(DCH):
                        nc.tensor.matmul(q_ps[:], lhsT=wt[:, k, 128 * jj:128 * (jj + 1)],
                                         rhs=xb[k][:], start=(k == 0), stop=(k == DCH - 1))
                    qraw = workA.tile([128, T], BF, tag="qraw", name="qraw")
                    nc.scalar.activation(qraw[:], q_ps[:], Copy)
                    qsw_ps = pst(128, T, "qsw", tag="m")
                    nc.tensor.matmul(qsw_ps[:], lhsT=pswap[:], rhs=qraw[:],
                                     start=True, stop=True)
                    sqq = workA.tile([128, T], F16, tag="sqq", name="sqq")
                    nc.scalar.activation(sqq[:], q_ps[:], Square)
                    nc.tensor.matmul(ssqq_ps[:], lhsT=bdall[:, mt, :],
                                     rhs=sqq[:], start=(mt == 0), stop=(mt == 7))
                    nc.vector.tensor_tensor(aq[mt][:], q_ps[:], rtq_c[:], MUL)
                    qsw_m = workA.tile([128, T], FP, tag="qswm", name="qswm")
                    nc.vector.tensor_tensor(qsw_m[:], qsw_ps[:], rtq_s[:], MUL)
                    nc.vector.tensor_tensor(aq[mt][:], aq[mt][:], qsw_m[:], ADD)
            qsc = batch_norm_scale(ssqq_ps, 16, rv2=rv2)
            for mt in range(DCH):
                rbch = pair_bcast(qsc, mt, rbc=rbc)
                nc.vector.tensor_tensor(qf[mt][:], aq[mt][:], rbch[:], MUL)

            # -- CA K (2 chunks over enc, width TK) --
            ksc_t = workB.tile([128, 2], FP, tag="ksc", name="ksc")
            nc.sync.dma_start(out=ksc_t[:],
                              in_=din[f'ca_kscale_{i}'].ap().rearrange("(t p) o -> p (t o)", p=128))
            wkt2 = wkv.tile([128, DCH, KVD], BF, tag="wkv", name="wkv")
            nc.sync.dma_start(out=wkt2[:],
                              in_=din[f'ca_wk_{i}'].ap().rearrange("(k p) n -> p k n", p=128))
            ssqc_ps = pst(128, TK, "ssqc", tag="o")
            ckraw = []
            for t in range(2):
                k_ps = pst(128, TK, "ckproj")
                for k in range(DCH):
                    nc.tensor.matmul(k_ps[:], lhsT=wkt2[:, k, 128 * t:128 * (t + 1)],
                                     rhs=encb[k][:], start=(k == 0), stop=(k == DCH - 1))
                kr = aqp.tile([128, TK], BF, tag=f"ckr{t}", name=f"ckr{t}")
                nc.scalar.activation(kr[:], k_ps[:], Copy)
                ckraw.append(kr)
                sqc = workA.tile([128, TK], F16, tag="sqc", name="sqc")
                nc.scalar.activation(sqc[:], k_ps[:], Square)
                nc.tensor.matmul(ssqc_ps[:], lhsT=bdall[:, t, :],
                                 rhs=sqc[:], start=(t == 0), stop=(t == 1))
            crr = batch_norm_scale(ssqc_ps, 4, width=TK)
            for t in range(2):
                rbch = pair_bcast(crr, t, nrows=4, width=TK)
                kh = workA.tile([128, TK], FP, tag="khca", name="khca")
                nc.vector.tensor_tensor(kh[:], ckraw[t][:], rbch[:], MUL)
                ckp = workB.tile([128, TK], BF, tag="ckp", name="ckp")
                nc.vector.tensor_scalar(
                    out=ckp[:], in0=kh[:],
                    scalar1=ksc_t[:, t:t + 1], scalar2=None, op0=MUL)
                for half in range(2):
                    kv = 2 * t + half
                    for dd in range(2):
                        nc.scalar.dma_start(out=kdca[kv][64 * dd:64 * (dd + 1), :],
                                            in_=ckp[64 * half:64 * (half + 1), :])

            # -- CA V (4 chunks of 128 enc tokens) --
            wvt2 = wkv.tile([128, DCH, KVD], BF, tag="wkv", name="wkv")
            nc.sync.dma_start(out=wvt2[:],
                              in_=din[f'ca_wv_{i}'].ap().rearrange("(k p) n -> p k n", p=128))
            for kv in range(KVH):
                nc.vector.memset(cv[kv][:, :, 64:65], 1.0)
                nc.vector.memset(cv[kv][:, :, 65:128], 0.0)
            for j in range(TP):
                v_ps = pst(128, KVD, "vproj", tag="m")
                for k in range(DCH):
                    nc.tensor.matmul(v_ps[:], lhsT=encb[k][:, 128 * j:128 * (j + 1)],
                                     rhs=wvt2[:, k, :], start=(k == 0), stop=(k == DCH - 1))
                for kv in range(KVH):
                    nc.scalar.activation(cv[kv][:, j, 0:64],
                                         v_ps[:, 64 * kv:64 * (kv + 1)], Copy)

            if DBG and i == 0:
                nc.sync.dma_start(out=dbg['tap_rbc'].ap(), in_=rbc[:])
                nc.sync.dma_start(out=dbg['tap_xb'].ap(), in_=xb[0][:])
                nc.sync.dma_start(out=dbg['tap_qf'].ap(), in_=qf[0][:])
            sa_attention(i)
            if DBG and i == 0:
                nc.sync.dma_start(out=dbg['tap_ao'].ap(), in_=ao[0][:])
            stream_out_proj(f'sa_wo_{i}')
            if DBG and i == 0:
                nc.sync.dma_start(out=dbg['tap_x1'].ap(), in_=x[0][:])

            # ---- LN2 (deferred) + CA ----
            rbc, rv2 = ln_prep()

            # -- CA Q (8 chunks, no rope) --
            ssqq_ps = pst(128, T, "ssqq", tag="o")
            qrawb = []
            for bki in range(2):
                wt = wbig.tile([128, DCH, 512], BF, tag="wbig", name="wbig")
                nc.sync.dma_start(
                    out=wt[:],
                    in_=din[f'ca_wq_{i}'].ap().rearrange("(k p) n -> p k n", p=128)
                    [:, :, bki * 512:(bki + 1) * 512])
                for jj in range(4):
                    mt = bki * 4 + jj
                    q_ps = pst(128, T, "qproj")
                    for k in range(DCH):
                        nc.tensor.matmul(q_ps[:], lhsT=wt[:, k, 128 * jj:128 * (jj + 1)],
                                         rhs=xb[k][:], start=(k == 0), stop=(k == DCH - 1))
                    qr = aqp.tile([128, T], BF, tag=f"cqr{mt}", name=f"cqr{mt}")
                    nc.scalar.activation(qr[:], q_ps[:], Copy)
                    qrawb.append(qr)
                    sqq = workA.tile([128, T], F16, tag="sqq", name="sqq")
                    nc.scalar.activation(sqq[:], q_ps[:], Square)
                    nc.tensor.matmul(ssqq_ps[:], lhsT=bdall[:, mt, :],
                                     rhs=sqq[:], start=(mt == 0), stop=(mt == 7))
            qsc = batch_norm_scale(ssqq_ps, 16, rv2=rv2)
            for mt in range(DCH):
                rbch = pair_bcast(qsc, mt, rbc=rbc)
                nc.vector.tensor_tensor(qf[mt][:], qrawb[mt][:], rbch[:], MUL)

            ca_attention(i)
            stream_out_proj(f'ca_wo_{i}')

            # ---- LN3 (deferred) + FFN ----
            rbc, rv2 = ln_prep()

            NF = F // 512
            for fb in range(NF):
                wgt = wbig.tile([128, DCH, 512], BF, tag="wbig", name="wbig")
                nc.sync.dma_start(
                    out=wgt[:],
                    in_=din[f'ffn_wg_{i}'].ap().rearrange("(k p) n -> p k n", p=128)
                    [:, :, fb * 512:(fb + 1) * 512])
                wut = wbig.tile([128, DCH, 512], BF, tag="wbig", name="wbig")
                nc.sync.dma_start(
                    out=wut[:],
                    in_=din[f'ffn_wu_{i}'].ap().rearrange("(k p) n -> p k n", p=128)
                    [:, :, fb * 512:(fb + 1) * 512])
                wdt = wbig.tile([128, 4, D], BF, tag="wbig", name="wbig")
                nc.sync.dma_start(
                    out=wdt[:],
                    in_=din[f'ffn_wd_{i}'].ap().rearrange("(k p) n -> p k n", p=128)
                    [:, fb * 4:(fb + 1) * 4, :])
                prods = []
                for hf in range(2):
                    gu = []
                    for which, wt in (('g', wgt), ('u', wut)):
                        g_ps = pst(128, 512, "s")
                        for jj in range(2):
                            j = 2 * hf + jj
                            for k in range(DCH):
                                nc.tensor.matmul(
                                    g_ps[:, 256 * jj:256 * (jj + 1)],
                                    lhsT=wt[:, k, 128 * j:128 * (j + 1)],
                                    rhs=xb[k][:], start=(k == 0), stop=(k == DCH - 1))
                        g_sb = ffnp.tile([128, 512], BF, tag=f"relu{which}", name=f"relu{which}")
                        if which == 'g':
                            nc.scalar.activation(g_sb[:], g_ps[:], Relu)
                        else:
                            nc.vector.tensor_scalar(out=g_sb[:], in0=g_ps[:],
                                                    scalar1=0.0, scalar2=None,
                                                    op0=AluOpType.max)
                        gu.append(g_sb)
                    pr = ffnp.tile([128, 512], BF, tag="prod", name="prod", bufs=3)
                    nc.vector.tensor_tensor(pr[:], gu[0][:], gu[1][:], MUL)
                    prods.append(pr)
                for m in range(DCH):
                    yp = pst(128, T, "yp", tag="m")
                    for kc in range(4):
                        nc.tensor.matmul(
                            yp[:],
                            lhsT=wdt[:, kc, 128 * m:128 * (m + 1)],
                            rhs=prods[kc // 2][:, 256 * (kc % 2):256 * (kc % 2 + 1)],
                            start=(kc == 0), stop=(kc == 3))
                    if fb == 0:
                        nc.scalar.activation(y_sb[m][:], yp[:], Copy)
                    else:
                        nc.vector.tensor_tensor(y_sb[m][:], y_sb[m][:], yp[:], ADD)
            for m in range(DCH):
                nc.vector.tensor_tensor(y_sb[m][:], y_sb[m][:], rv2[:], MUL)
                nc.vector.tensor_tensor(x[m][:], x[m][:], y_sb[m][:], ADD)

        # ---- final norm + output ----
        ssf = pst(128, T, "ssf", tag="o")
        for m in range(DCH):
            sq = workA.tile([128, T], F16, tag="sq", name="sq")
            nc.vector.tensor_tensor(sq[:], x[m][:], x[m][:], MUL)
            nc.tensor.matmul(ssf[:], lhsT=ones[:], rhs=sq[:],
                             start=(m == 0), stop=(m == DCH - 1))
        lnf = workB.tile([128, T], FP, tag="lnf", name="lnf")
        nc.scalar.activation(lnf[:], ssf[:], Ln, bias=eps_t[:], scale=1.0 / D)
        rbcf = workA.tile([128, T], FP, tag="rbcf", name="rbcf")
        nc.scalar.activation(rbcf[:], lnf[:], Exp, scale=-0.5)
        for m in range(DCH):
            ot = workB.tile([128, T], FP, tag="otile", name="otile", bufs=2)
            nc.vector.tensor_tensor(ot[:], x[m][:], rbcf[:], MUL)
            nc.vector.tensor_scalar(out=ot[:], in0=ot[:],
                                    scalar1=fscale[:, m:m + 1], scalar2=None, op0=MUL)
            nc.sync.dma_start(out=out_dram.ap()[128 * m:128 * (m + 1), :], in_=ot[:])

    nc.compile()
    return nc


def _get_program():
    global _PROG
    if _PROG is None:
        _PROG = _build_program()
    return _PROG


def kernel(**inputs):
    from concourse import bass_utils
    host, per_core = host_prepare(inputs)
    nc = _get_program()
    in_maps = []
    for c in range(NCORES):
        m = dict(per_core[c])
        for k, v in host.items():
            if k.startswith('rope'):
                continue  # per-core sliced versions already present
            m[k] = v
        in_maps.append(m)
    res = bass_utils.run_bass_kernel_spmd(nc, in_maps, list(range(NCORES)))
    out = np.empty((B, TQ, D), np.float32)
    for c in range(NCORES):
        grp, r = c // TP, c % TP
        out[grp, r::TP] = res.results[c]['outT'].T
    return out



# revision 51
# speedup vs baseline: 1.1233x; 1.0035x over previous
"""Self-contained Trainium2 Bass kernel for nn_EncoderDecoderTransformer_90941637525663.

Sequence-parallel over 8 NeuronCores (2 batch groups x 4 token shards), with
INTERLEAVED token assignment (core r owns tokens t = 4j + r): causal
visibility becomes structurally uniform across cores, so each head-pair
processes 12 score blocks instead of 20 (full blocks need no mask; the four
diagonal blocks use a per-core triangular mask loaded as input). Activations
live transposed in SBUF (feature dim on partitions, tokens on free dim).
Per layer one AllGather exchanges K (normalized + roped) and V (pre-scaled
by the sender 1/rms row), with Q projection and cross-attention K/V filling
the gather latency. All heavy matmuls run in bf16; helper selector and
accumulator weights (ones/bdall/sel16/squares/scales) are fp16 so they stay
on the fast full-width weight path. All rms factors use exp(-0.5*ln(mean+eps))
so every activation lives in one ACT table set (a single table load total,
vs ~1.3us per sqrt<->exp switch). Softmax row-sum division is batched per
head pair: vector reciprocal + gpsimd partition-broadcast. LN scales are
folded into consumer weights host-side; deferred-LN per-head rms folds rinv
in via rv2.
"""
import sys
sys.path.insert(0, '/opt/trn_rl_repo')
import numpy as np

B, TQ, TK, D, H, KVH, L, F = 2, 1024, 512, 1024, 16, 4, 2, 4096
HD, KVD = 64, 256
EPS = 1e-6
NCORES, TP = 8, 4
T = TQ // TP           # 256 tokens per core (interleaved: t = 4j + r)
DCH = D // 128         # 8 feature chunks
NEG = -3.0e38
AGR = 2 * T            # ag rows: [0,T)=k (normalized), [T,2T)=v (rinv-scaled)


def _bf16():
    import ml_dtypes
    return ml_dtypes.bfloat16


def _rope_tables(Tlen, hd, theta=10000.0):
    freqs = 1.0 / theta ** (np.arange(0, hd, 2, dtype=np.float32) / hd)
    ang = np.outer(np.arange(Tlen, dtype=np.float32), freqs)
    return np.cos(ang).astype(np.float32), np.sin(ang).astype(np.float32)


def host_prepare(inputs):
    """Returns (host, per_core): folded shared arrays + per-core arrays."""
    bf16 = _bf16()
    inputs = {k: np.ascontiguousarray(np.asarray(v, dtype=np.float32))
              for k, v in inputs.items()}
    cos_f, sin_f = _rope_tables(TQ, HD)       # [TQ, 32]

    def bf(a):
        return np.ascontiguousarray(a.astype(bf16))

    fp16 = np.float16

    host = {}
    for i in range(L):
        ln1 = (1.0 + inputs['ln1_s'][i])[:, None]
        ln2 = (1.0 + inputs['ln2_s'][i])[:, None]
        ln3 = (1.0 + inputs['ln3_s'][i])[:, None]
        host[f'sa_wq_{i}'] = bf(ln1 * inputs['sa_wq'][i])
        host[f'sa_wk_{i}'] = bf(ln1 * inputs['sa_wk'][i])
        host[f'sa_wv_{i}'] = bf(ln1 * inputs['sa_wv'][i])
        host[f'sa_wo_{i}'] = bf(inputs['sa_wo'][i])
        host[f'ca_wq_{i}'] = bf(ln2 * inputs['ca_wq'][i])
        host[f'ca_wk_{i}'] = bf(inputs['ca_wk'][i])
        host[f'ca_wv_{i}'] = bf(inputs['ca_wv'][i])
        host[f'ca_wo_{i}'] = bf(inputs['ca_wo'][i])
        host[f'ffn_wg_{i}'] = bf(ln3 * inputs['ffn_wg'][i])
        host[f'ffn_wu_{i}'] = bf(ln3 * inputs['ffn_wu'][i])
        host[f'ffn_wd_{i}'] = bf(inputs['ffn_wd'][i])
        # rope tables with (1+qn)/(1+kn) head-dim scales folded; [128, TQ]
        for which, dvec in [('q', inputs['sa_qn'][i]), ('k', inputs['sa_kn'][i])]:
            d1, d2 = 1.0 + dvec[:32], 1.0 + dvec[32:]
            C = np.concatenate([d1[:, None] * cos_f.T, d2[:, None] * cos_f.T], 0)
            S = np.concatenate([-d2[:, None] * sin_f.T, d1[:, None] * sin_f.T], 0)
            host[f'rope{which}_c_{i}'] = np.ascontiguousarray(np.concatenate([C, C], 0))
            host[f'rope{which}_s_{i}'] = np.ascontiguousarray(np.concatenate([S, S], 0))
        sc = ((1.0 + inputs['ca_qn'][i]) * (1.0 + inputs['ca_kn'][i])).astype(np.float32)
        host[f'ca_kscale_{i}'] = np.tile(sc, KVH)[:, None].copy()   # [256, 1]
    host['final_scale'] = (1.0 + inputs['final_s'])[:, None].copy()  # [D, 1]
    # bdall[:, mt, :]: [128,128] stationary (fp16, M padded to 128 for fast
    # weight load) accumulating chunk mt's per-head-pair row sums into rows
    # 2mt/2mt+1 of a [128,T] psum (other rows zero)
    bdall = np.zeros((128, 8, 128), fp16)
    for mt in range(8):
        bdall[0:64, mt, 2 * mt] = 1.0
        bdall[64:128, mt, 2 * mt + 1] = 1.0
    host['bdall'] = bdall
    # sel16[0:n, mt, :]: [n,128] stationary (fp16) broadcasting rows 2mt/2mt+1
    # of an [n,T] scale tile to the 64-row head halves of a [128,T] output
    sel16 = np.zeros((16, 8, 128), fp16)
    for mt in range(8):
        sel16[2 * mt, mt, 0:64] = 1.0
        sel16[2 * mt + 1, mt, 64:128] = 1.0
    host['sel16'] = sel16
    # rope half-swap permutation (within each 64-row head block): [128, 128]
    psw = np.zeros((128, 128), np.float32)
    for base in (0, 64):
        for r in range(32):
            psw[base + 32 + r, base + r] = 1.0      # out row base+r <- in row base+32+r
            psw[base + r, base + 32 + r] = 1.0      # out row base+32+r <- in row base+r
    host['pswap'] = bf(psw)

    per_core = []
    kl = np.arange(128)[:, None]
    ql = np.arange(128)[None, :]
    tri_inc = np.where(kl <= ql, 0.0, NEG).astype(np.float32)
    tri_exc = np.where(kl < ql, 0.0, NEG).astype(np.float32)
    for c in range(NCORES):
        grp, r = c // TP, c % TP
        tok = slice(r, None, TP)   # interleaved tokens: 4j + r
        pc = {
            'xT': np.ascontiguousarray(inputs['x'][grp].T[:, tok]),
            'encTb': bf(inputs['encoder_out'][grp].T),
        }
        # diagonal-chunk additive mask vs source core g: k-token 4p+g visible
        # to q-token 4j+r within the same 128-block iff p<=j (g<=r) / p<j (g>r)
        dm = np.empty((128, TP, 128), np.float32)
        for g in range(TP):
            dm[:, g, :] = tri_inc if g <= r else tri_exc
        pc['dmask'] = np.ascontiguousarray(dm)
        for i in range(L):
            for nm in ('ropeq_c', 'ropeq_s', 'ropek_c', 'ropek_s'):
                pc[f'{nm}_{i}'] = np.ascontiguousarray(host[f'{nm}_{i}'][:, tok])
        per_core.append(pc)
    return host, per_core


_PROG = None
DBG = False


def _build_program(nlayers=L):
    import concourse.bass as bass
    import concourse.tile as tile
    from concourse import bacc, mybir
    from concourse.alu_op_type import AluOpType
    from contextlib import ExitStack

    R = mybir.dt.float32r
    FP = mybir.dt.float32
    BF = mybir.dt.bfloat16
    F16 = mybir.dt.float16
    Exp = mybir.ActivationFunctionType.Exp
    Ln = mybir.ActivationFunctionType.Ln
    Relu = mybir.ActivationFunctionType.Relu
    Square = mybir.ActivationFunctionType.Square
    Copy = mybir.ActivationFunctionType.Copy

    class _Bacc(bacc.Bacc):
        def insert_act_table_loads(self):
            # Force exp/ln (this kernel's only table-bound activations) to
            # resolve to the combined natural_log_exp_and_others set: the
            # default first-match selection alternates exp_and_others /
            # natural_log and pays a ~1.3us table load per switch.
            import bass_rust as _bass_rust
            has_activation = any(
                isinstance(ins, mybir.InstActivation)
                for b in self.main_func.blocks
                for ins in b.instructions
            )
            if not has_activation:
                return
            from concourse.hw_specs import get_activation_tables
            tables = list(get_activation_tables(self.m.arch).items())
            combined = next(i for i, (nm, _) in enumerate(tables)
                            if nm == 'natural_log_exp_and_others')
            tables = [(nm, s if i == combined else set())
                      for i, (nm, s) in enumerate(tables)]
            _bass_rust.insert_act_table_loads(self, tables)

    nc = _Bacc("TRN2", target_bir_lowering=False, debug=False,
               num_devices=NCORES)

    din = {}
    def dri(name, shape, dt):
        din[name] = nc.dram_tensor(name, list(shape), dt, kind="ExternalInput")

    dri('xT', (D, T), FP)
    dri('encTb', (D, TK), BF)
    dri('dmask', (128, TP, 128), FP)
    dri('final_scale', (D, 1), FP)
    dri('bdall', (128, 8, 128), F16)
    dri('sel16', (16, 8, 128), F16)
    dri('pswap', (128, 128), BF)
    for i in range(nlayers):
        dri(f'sa_wq_{i}', (D, D), BF); dri(f'sa_wk_{i}', (D, KVD), BF)
        dri(f'sa_wv_{i}', (D, KVD), BF); dri(f'sa_wo_{i}', (D, D), BF)
        dri(f'ca_wq_{i}', (D, D), BF); dri(f'ca_wk_{i}', (D, KVD), BF)
        dri(f'ca_wv_{i}', (D, KVD), BF); dri(f'ca_wo_{i}', (D, D), BF)
        dri(f'ffn_wg_{i}', (D, F), BF); dri(f'ffn_wu_{i}', (D, F), BF)
        dri(f'ffn_wd_{i}', (F, D), BF)
        for nm in ('ropeq_c', 'ropeq_s', 'ropek_c', 'ropek_s'):
            dri(f'{nm}_{i}', (128, T), FP)
        dri(f'ca_kscale_{i}', (KVD, 1), FP)
    out_dram = nc.dram_tensor('outT', [D, T], FP, kind="ExternalOutput")
    dbg = {}
    if DBG:
        for nm, shp, dt in [('tap_rbc', [128, T], FP), ('tap_xb', [128, T], BF),
                            ('tap_qsc', [16, T], FP), ('tap_qf', [128, T], BF),
                            ('tap_kd', [128, T], BF), ('tap_vown', [128, 256], BF),
                            ('tap_kfull', [128, TQ], BF), ('tap_vfull', [128, TP * 2 * 128], BF),
                            ('tap_rvg', [128, TP * 2], BF), ('tap_ao', [128, T], BF),
                            ('tap_x1', [128, T], FP)]:
            dbg[nm] = nc.dram_tensor(nm, shp, dt, kind="ExternalOutput")
    ag_in = [nc.dram_tensor(f'ag_in_{i}', [AGR, T], BF) for i in range(nlayers)]
    ag_out = [nc.dram_tensor(f'ag_out_{i}', [AGR * TP, T], BF) for i in range(nlayers)]
    agw_in = nc.dram_tensor('agw_in', [1, 128], BF)
    agw_out = nc.dram_tensor('agw_out', [TP, 128], BF)
    GROUPS = [[0, 1, 2, 3], [4, 5, 6, 7]]

    with nc.allow_low_precision(reason="bf16 pipeline"), \
            tile.TileContext(nc) as tc, ExitStack() as ctx:
        consts = ctx.enter_context(tc.tile_pool(name="consts", bufs=1))
        state = ctx.enter_context(tc.tile_pool(name="state", bufs=1))
        aqp = ctx.enter_context(tc.tile_pool(name="aqp", bufs=1))
        kvf = ctx.enter_context(tc.tile_pool(name="kvf", bufs=2))
        wbig = ctx.enter_context(tc.tile_pool(name="wbig", bufs=3))
        wkv = ctx.enter_context(tc.tile_pool(name="wkv", bufs=2))
        workA = ctx.enter_context(tc.tile_pool(name="workA", bufs=2))
        workB = ctx.enter_context(tc.tile_pool(name="workB", bufs=2))
        psb = ctx.enter_context(tc.tile_pool(name="psb", bufs=4))
        ffnp = ctx.enter_context(tc.tile_pool(name="ffnp", bufs=2))
        ropep = ctx.enter_context(tc.tile_pool(name="ropep", bufs=2))
        ps = ctx.enter_context(tc.tile_pool(name="ps", bufs=8, space="PSUM"))

        def pst(p_, f_, name, tag="s"):
            bufs = {"s": 3, "o": 2, "m": 3}[tag]
            return ps.tile([p_, f_], FP, tag=f"ps_{tag}", name=name, bufs=bufs)

        MUL, ADD = AluOpType.mult, AluOpType.add

        # ---- warm-up collective: the first collective's entry barrier
        # absorbs inter-core launch skew; run it on a tiny buffer up front so
        # the wait overlaps constant/weight loads and the pre-AG compute ----
        warm = consts.tile([1, 128], BF, tag="warm", name="warm")
        nc.vector.memset(warm[:], 0.0)
        nc.sync.dma_start(out=agw_in.ap(), in_=warm[:])
        nc.gpsimd.collective_compute(
            "AllGather", mybir.AluOpType.bypass, replica_groups=GROUPS,
            ins=[agw_in.ap().opt()], outs=[agw_out.ap().opt()])

        # ---- constants ----
        ones = consts.tile([128, 128], F16, tag="ones", name="ones")
        nc.vector.memset(ones[:], 1.0)
        bdall = consts.tile([128, 8, 128], F16, tag="bdall", name="bdall")
        nc.sync.dma_start(out=bdall[:], in_=din['bdall'].ap())
        sel16 = consts.tile([16, 8, 128], F16, tag="sel16", name="sel16")
        nc.sync.dma_start(out=sel16[:], in_=din['sel16'].ap())
        eps_t = consts.tile([128, 1], FP, tag="eps", name="eps")
        nc.vector.memset(eps_t[:], EPS)
        id1 = consts.tile([1, 1], FP, tag="id1", name="id1")
        nc.vector.memset(id1[:], 1.0)
        pswap = consts.tile([128, 128], BF, tag="pswap", name="pswap")
        nc.sync.dma_start(out=pswap[:], in_=din['pswap'].ap())
        dmask = consts.tile([128, TP, 128], FP, tag="dmask", name="dmask")
        nc.sync.dma_start(out=dmask[:], in_=din['dmask'].ap())
        fscale = consts.tile([128, DCH], FP, tag="fscale", name="fscale")
        nc.sync.dma_start(out=fscale[:],
                          in_=din['final_scale'].ap().rearrange("(k p) o -> p (k o)", p=128))

        # ---- persistent state ----
        x = [state.tile([128, T], FP, tag=f"x{m}", name=f"x{m}") for m in range(DCH)]
        for m in range(DCH):
            nc.sync.dma_start(out=x[m][:], in_=din['xT'].ap()[128 * m:128 * (m + 1), :])
        xb = [state.tile([128, T], BF, tag=f"xb{m}", name=f"xb{m}") for m in range(DCH)]
        encb = [state.tile([128, TK], BF, tag=f"enc{m}", name=f"enc{m}") for m in range(DCH)]
        for m in range(DCH):
            nc.sync.dma_start(out=encb[m][:], in_=din['encTb'].ap()[128 * m:128 * (m + 1), :])
        ao = [state.tile([128, T], BF, tag=f"ao{m}", name=f"ao{m}") for m in range(DCH)]
        qf = [state.tile([128, T], BF, tag=f"qf{t}", name=f"qf{t}") for t in range(H // 2)]
        kdca = [state.tile([128, TK], BF, tag=f"kdca{k}", name=f"kdca{k}") for k in range(KVH)]
        cv = [state.tile([128, TP, 128], BF, tag=f"cv{k}", name=f"cv{k}") for k in range(KVH)]
        y_sb = [state.tile([128, T], FP, tag=f"ysb{m}", name=f"ysb{m}") for m in range(DCH)]
        aq = y_sb

        # ---------------- helpers ----------------
        def ln_prep(tagp=""):
            """rms over D partitions of fp32 x -> (rbc [128,T] FP broadcast of
            1/rms, rv2 [128,T] FP = rbc^2). Also refreshes xb (bf16 cast of x).
            Uses exp(-0.5*ln(mean+eps)) on full 128 rows so no partition
            broadcast is needed (ACT cost is free-dim bound)."""
            ss = pst(128, T, "ss", tag="o")
            for m in range(DCH):
                sq = workA.tile([128, T], F16, tag="sq", name="sq")
                nc.vector.tensor_tensor(sq[:], x[m][:], x[m][:], MUL)
                nc.tensor.matmul(ss[:], lhsT=ones[:], rhs=sq[:],
                                 start=(m == 0), stop=(m == DCH - 1))
                nc.scalar.activation(xb[m][:], x[m][:], Copy)
            lnv = workA.tile([128, T], FP, tag="lnv" + tagp, name="lnv")
            nc.scalar.activation(lnv[:], ss[:], Ln, bias=eps_t[:], scale=1.0 / D)
            rbc = workA.tile([128, T], FP, tag="rbc" + tagp, name="rbc")
            nc.scalar.activation(rbc[:], lnv[:], Exp, scale=-0.5)
            rv2 = workA.tile([128, T], FP, tag="rv2" + tagp, name="rv2")
            nc.vector.tensor_tensor(rv2[:], rbc[:], rbc[:], MUL)
            return rbc, rv2

        def batch_norm_scale(ssq_ps, nrows, rv2=None, width=T):
            """Per-head-pair raw sums [nrows, width] psum -> 1/rms scale (F16)
            via exp(-0.5*ln(mean+eps)). With rv2: deferred-LN variant (sums
            scaled by rinv^2 first)."""
            if rv2 is not None:
                tk = workA.tile([nrows, width], FP, tag="bnt", name="bnt",
                                padded_shape=[16, TK])
                nc.vector.tensor_tensor(tk[:], ssq_ps[0:nrows, 0:width],
                                        rv2[0:nrows, :width], MUL)
                src = tk[:]
            else:
                src = ssq_ps[0:nrows, 0:width]
            sr = workB.tile([nrows, width], FP, tag="bns", name="bns",
                            padded_shape=[16, TK])
            nc.scalar.activation(sr[:], src, Ln, bias=eps_t[0:nrows, :], scale=1.0 / HD)
            rr = workB.tile([nrows, width], F16, tag="bnr", name="bnr",
                            padded_shape=[16, TK])
            nc.scalar.activation(rr[:], sr[:], Exp, scale=-0.5)
            return rr

        def pair_bcast(scale_tile, t, nrows=16, width=T, rbc=None):
            """Broadcast rows 2t/2t+1 of an [nrows, width] F16 scale tile to the
            64-row halves of a [128, width] FP tile via PE selector matmul.
            If rbc given, fold the per-token rinv into the drain."""
            bc_ps = pst(128, width, "bch", tag="m")
            nc.tensor.matmul(bc_ps[:], lhsT=sel16[0:nrows, t, :],
                             rhs=scale_tile[:],
                             start=True, stop=True)
            rbch = workA.tile([128, width], FP, tag="rbch", name="rbch",
                              padded_shape=[128, TK])
            if rbc is not None:
                nc.vector.tensor_tensor(rbch[:], bc_ps[:], rbc[:, :width], MUL)
            else:
                nc.scalar.activation(rbch[:], bc_ps[:], Copy)
            return rbch

        def norm_pair(qt, o2):
            """Divide both heads' outputs by their softmax rowsums (row 64 of
            each o2): recip on vector, partition-broadcast on the otherwise
            idle gpsimd, final muls on vector."""
            rs2 = workB.tile([1, 2, T], FP, tag="rs2", name="rs2", bufs=3)
            nc.scalar.activation(rs2[0:1, 0, :], o2[0][64:65, :], Copy)
            nc.vector.tensor_copy(rs2[0:1, 1, :], o2[1][64:65, :])
            rr2 = workB.tile([1, 2, T], FP, tag="rr2", name="rr2", bufs=3)
            nc.vector.reciprocal_approx_fast(rr2[:], rs2[:])
            bc_sb = workB.tile([128, 2, T], FP, tag="bc_sb", name="bc_sb", bufs=3)
            nc.gpsimd.partition_broadcast(bc_sb[:], rr2[:])
            for h, par in ((0, 0), (1, 64)):
                nc.vector.tensor_tensor(ao[qt][par:par + 64, :],
                                        o2[h][0:64, :],
                                        bc_sb[par:par + 64, h, :], MUL)

        def sa_attention(i):
            ag = ag_out[i].ap()
            for kv in range(KVH):
                kga = kvf.tile([128, TP, T], BF, tag="kfull", name="kfull")
                src = bass.AP(tensor=ag.tensor, offset=(64 * kv) * T,
                              ap=[[T, 64], [AGR * T, TP], [1, T]])
                for dd in range(2):
                    nc.sync.dma_start(out=kga[64 * dd:64 * (dd + 1), :, :], in_=src)
                vga = kvf.tile([128, TP, 2, 128], BF, tag="vfull", name="vfull")
                nc.vector.memset(vga[:, :, :, 64:65], 1.0)
                nc.vector.memset(vga[:, :, :, 65:128], 0.0)
                for cb in range(2):
                    vap = bass.AP(tensor=ag.tensor,
                                  offset=T * T + 128 * T * cb + 64 * kv,
                                  ap=[[T, 128], [AGR * T, TP], [1, 64]])
                    nc.sync.dma_start(out=vga[:, :, cb, 0:64], in_=vap)
                for pp in range(2):
                    qt = 2 * kv + pp
                    o2 = [pst(128, T, f"o{h}", tag="o") for h in range(2)]
                    mi = 0
                    # C=0 chunks [128k x 256q]: left half (vs J0) is the
                    # diagonal (masked), right half (vs J1) fully visible
                    for b in range(2):
                        s2t = [pst(128, 512, "s") for _ in range(2)]
                        for gg in range(2):
                            g = 2 * b + gg
                            for h, par in ((0, 0), (1, 64)):
                                nc.tensor.matmul(
                                    s2t[h][:, 256 * gg:256 * (gg + 1)],
                                    lhsT=kga[par:par + 64, g, 0:128],
                                    rhs=qf[qt][par:par + 64, :],
                                    start=True, stop=True)
                        p2t = []
                        for h in range(2):
                            sv = s2t[h][:].rearrange("p (c t) -> p c t", c=2)
                            nc.vector.tensor_tensor(
                                sv[:, :, 0:128], sv[:, :, 0:128],
                                dmask[:, 2 * b:2 * b + 2, :], ADD)
                            p_sb = psb.tile([128, 2, T], BF, tag="p_sb", name="p_sb")
                            nc.scalar.activation(p_sb[:], sv, Exp, scale=0.125)
                            p2t.append(p_sb)
                        for gg in range(2):
                            g = 2 * b + gg
                            for h in range(2):
                                nc.tensor.matmul(o2[h][:], lhsT=vga[:, g, 0, :],
                                                 rhs=p2t[h][:, gg, :],
                                                 start=(mi == 0), stop=False,
                                                 skip_group_check=True)
                            mi += 1
                    # C=1 chunks [128k x 128q]: all diagonal, right half only
                    s2t = [pst(128, 512, "s") for _ in range(2)]
                    for h, par in ((0, 0), (1, 64)):
                        for g in range(TP):
                            nc.tensor.matmul(
                                s2t[h][:, 128 * g:128 * (g + 1)],
                                lhsT=kga[par:par + 64, g, 128:256],
                                rhs=qf[qt][par:par + 64, 128:256],
                                start=True, stop=True)
                    p2t = []
                    for h in range(2):
                        sv = s2t[h][:].rearrange("p (g t) -> p g t", g=TP)
                        nc.vector.tensor_tensor(sv, sv, dmask[:], ADD)
                        p_sb = psb.tile([128, TP, 128], BF, tag="p_sb", name="p_sb")
                        nc.scalar.activation(p_sb[:], sv, Exp, scale=0.125)
                        p2t.append(p_sb)
                    for g in range(TP):
                        for h in range(2):
                            nc.tensor.matmul(o2[h][:, 128:256], lhsT=vga[:, g, 1, :],
                                             rhs=p2t[h][:, g, :],
                                             start=False, stop=(g == TP - 1),
                                             skip_group_check=True)
                    norm_pair(qt, o2)

        def ca_attention(i):
            nk_chunks = TK // 128
            for kv in range(KVH):
                ksrc = kdca[kv]
                vsrc = cv[kv]
                for pp in range(2):
                    # process both heads of pair tile qt together: their score
                    # matmuls use disjoint PE row groups (lhsT base 0 vs 64) and
                    # overlap when issued back-to-back
                    qt = 2 * kv + pp
                    o2 = [pst(128, T, f"o{h}", tag="o") for h in range(2)]
                    mi = 0
                    for half in range(nk_chunks // 2):
                        s2t = [pst(128, 512, "s") for _ in range(2)]
                        for cc in range(2):
                            c = 2 * half + cc
                            for h, par in ((0, 0), (1, 64)):
                                nc.tensor.matmul(
                                    s2t[h][:, 256 * cc:256 * (cc + 1)],
                                    lhsT=ksrc[par:par + 64, 128 * c:128 * (c + 1)],
                                    rhs=qf[qt][par:par + 64, :], start=True, stop=True)
                        p2t = []
                        for h in range(2):
                            p_sb = psb.tile([128, 2, T], BF, tag="p_sb", name="p_sb")
                            nc.scalar.activation(p_sb[:], s2t[h][:].rearrange(
                                "p (c t) -> p c t", c=2), Exp, scale=0.125)
                            p2t.append(p_sb)
                        for cc in range(2):
                            c = 2 * half + cc
                            for h in range(2):
                                nc.tensor.matmul(o2[h][:], lhsT=vsrc[:, c, :],
                                                 rhs=p2t[h][:, cc, :],
                                                 start=(mi == 0),
                                                 stop=(mi == nk_chunks - 1),
                                                 skip_group_check=True)
                            mi += 1
                    norm_pair(qt, o2)

        def stream_out_proj(w_name):
            for bki in range(2):
                wt = wbig.tile([128, DCH, 512], BF, tag="wbig", name="wbig")
                nc.sync.dma_start(
                    out=wt[:],
                    in_=din[w_name].ap().rearrange("(k p) n -> p k n", p=128)
                    [:, :, bki * 512:(bki + 1) * 512])
                for j in range(4):
                    m = bki * 4 + j
                    y_ps = pst(128, T, "proj", tag="m")
                    for k in range(DCH):
                        nc.tensor.matmul(y_ps[:], lhsT=wt[:, k, 128 * j:128 * (j + 1)],
                                         rhs=ao[k][:],
                                         start=(k == 0), stop=(k == DCH - 1))
                    nc.vector.tensor_tensor(x[m][:], x[m][:], y_ps[:], ADD)

        # ================= layers =================
        for i in range(nlayers):
            # ---- LN1 (deferred) + SA ----
            rbc, rv2 = ln_prep()
            rvT = []
            for j in range(2):
                tp_ps = pst(128, 1, "tp", tag="m")
                nc.tensor.transpose(tp_ps[:], rbc[0:1, 128 * j:128 * (j + 1)], id1[:])
                rv_b = workB.tile([128, 1], FP, tag=f"rvT{j}", name=f"rvT{j}")
                nc.scalar.activation(rv_b[:], tp_ps[:], Copy)
                rvT.append(rv_b)

            rtq_c = ropep.tile([128, T], FP, tag="rtc", name="rtc")
            nc.sync.dma_start(out=rtq_c[:], in_=din[f'ropeq_c_{i}'].ap())
            rtq_s = ropep.tile([128, T], FP, tag="rts", name="rts")
            nc.sync.dma_start(out=rtq_s[:], in_=din[f'ropeq_s_{i}'].ap())
            rtk_c = ropep.tile([128, T], FP, tag="rtc", name="rtc")
            nc.sync.dma_start(out=rtk_c[:], in_=din[f'ropek_c_{i}'].ap())
            rtk_s = ropep.tile([128, T], FP, tag="rts", name="rts")
            nc.sync.dma_start(out=rtk_s[:], in_=din[f'ropek_s_{i}'].ap())

            # -- SA K (2 chunks of 128 cols) --
            wkt = wkv.tile([128, DCH, KVD], BF, tag="wkv", name="wkv")
            nc.sync.dma_start(out=wkt[:],
                              in_=din[f'sa_wk_{i}'].ap().rearrange("(k p) n -> p k n", p=128))
            ssqk_ps = pst(128, T, "ssqk", tag="o")
            ka = []
            for t in range(2):
                k_ps = pst(128, T, "kproj")
                for k in range(DCH):
                    nc.tensor.matmul(k_ps[:], lhsT=wkt[:, k, 128 * t:128 * (t + 1)],
                                     rhs=xb[k][:], start=(k == 0), stop=(k == DCH - 1))
                kraw = workA.tile([128, T], BF, tag="kraw", name="kraw")
                nc.scalar.activation(kraw[:], k_ps[:], Copy)
                ksw_ps = pst(128, T, "ksw", tag="m")
                nc.tensor.matmul(ksw_ps[:], lhsT=pswap[:], rhs=kraw[:],
                                 start=True, stop=True)
                sqk = workA.tile([128, T], F16, tag="sqk", name="sqk")
                nc.scalar.activation(sqk[:], k_ps[:], Square)
                nc.tensor.matmul(ssqk_ps[:], lhsT=bdall[:, t, :],
                                 rhs=sqk[:], start=(t == 0), stop=(t == 1))
                kat = workB.tile([128, T], FP, tag=f"ka{t}", name=f"ka{t}")
                nc.vector.tensor_tensor(kat[:], k_ps[:], rtk_c[:], MUL)
                ksw_m = workA.tile([128, T], FP, tag="kswm", name="kswm")
                nc.vector.tensor_tensor(ksw_m[:], ksw_ps[:], rtk_s[:], MUL)
                nc.vector.tensor_tensor(kat[:], kat[:], ksw_m[:], ADD)
                ka.append(kat)
            ksc = batch_norm_scale(ssqk_ps, 4, rv2=rv2)
            for t in range(2):
                rbch = pair_bcast(ksc, t, nrows=4, rbc=rbc)
                kf = workA.tile([128, T], BF, tag="kf", name="kf")
                nc.vector.tensor_tensor(kf[:], ka[t][:], rbch[:], MUL)
                nc.sync.dma_start(out=ag_in[i].ap()[128 * t:128 * (t + 1), :],
                                  in_=kf[:])

            # -- SA V (2 token chunks of 128), rinv-scaled before the gather --
            wvt = wkv.tile([128, DCH, KVD], BF, tag="wkv", name="wkv")
            nc.sync.dma_start(out=wvt[:],
                              in_=din[f'sa_wv_{i}'].ap().rearrange("(k p) n -> p k n", p=128))
            for j in range(2):
                v_ps = pst(128, KVD, "vproj", tag="m")
                for k in range(DCH):
                    nc.tensor.matmul(v_ps[:], lhsT=xb[k][:, 128 * j:128 * (j + 1)],
                                     rhs=wvt[:, k, :], start=(k == 0), stop=(k == DCH - 1))
                vraw = workA.tile([128, KVD], BF, tag="vraw", name="vraw")
                nc.vector.tensor_scalar(out=vraw[:], in0=v_ps[:],
                                        scalar1=rvT[j][:], scalar2=None, op0=MUL)
                nc.sync.dma_start(out=ag_in[i].ap()[T + 128 * j:T + 128 * (j + 1), :],
                                  in_=vraw[:])

            nc.gpsimd.collective_compute(
                "AllGather", mybir.AluOpType.bypass, replica_groups=GROUPS,
                ins=[ag_in[i].ap().opt()], outs=[ag_out[i].ap().opt()])

            # -- SA Q (8 chunks) --
            ssqq_ps = pst(128, T, "ssqq", tag="o")
            for bki in range(2):
                wt = wbig.tile([128, DCH, 512], BF, tag="wbig", name="wbig")
                nc.sync.dma_start(
                    out=wt[:],
                    in_=din[f'sa_wq_{i}'].ap().rearrange("(k p) n -> p k n", p=128)
                    [:, :, bki * 512:(bki + 1) * 512])
                for jj in range(4):
                    mt = bki * 4 + jj
                    q_ps = pst(128, T, "qproj")
                    for k in range(DCH):
                        nc.tensor.matmul(q_ps[:], lhsT=wt[:, k, 128 * jj:128 * (jj + 1)],
                                         rhs=xb[k][:], start=(k == 0), stop=(k == DCH - 1))
                    qraw = workA.tile([128, T], BF, tag="qraw", name="qraw")
                    nc.scalar.activation(qraw[:], q_ps[:], Copy)
                    qsw_ps = pst(128, T, "qsw", tag="m")
                    nc.tensor.matmul(qsw_ps[:], lhsT=pswap[:], rhs=qraw[:],
                                     start=True, stop=True)
                    sqq = workA.tile([128, T], F16, tag="sqq", name="sqq")
                    nc.scalar.activation(sqq[:], q_ps[:], Square)
                    nc.tensor.matmul(ssqq_ps[:], lhsT=bdall[:, mt, :],
                                     rhs=sqq[:], start=(mt == 0), stop=(mt == 7))
                    nc.vector.tensor_tensor(aq[mt][:], q_ps[:], rtq_c[:], MUL)
                    qsw_m = workA.tile([128, T], FP, tag="qswm", name="qswm")
                    nc.vector.tensor_tensor(qsw_m[:], qsw_ps[:], rtq_s[:], MUL)
                    nc.vector.tensor_tensor(aq[mt][:], aq[mt][:], qsw_m[:], ADD)
            qsc = batch_norm_scale(ssqq_ps, 16, rv2=rv2)
            for mt in range(DCH):
                rbch = pair_bcast(qsc, mt, rbc=rbc)
                nc.vector.tensor_tensor(qf[mt][:], aq[mt][:], rbch[:], MUL)

            # -- CA K (2 chunks over enc, width TK) --
            ksc_t = workB.tile([128, 2], FP, tag="ksc", name="ksc")
            nc.sync.dma_start(out=ksc_t[:],
                              in_=din[f'ca_kscale_{i}'].ap().rearrange("(t p) o -> p (t o)", p=128))
            wkt2 = wkv.tile([128, DCH, KVD], BF, tag="wkv", name="wkv")
            nc.sync.dma_start(out=wkt2[:],
                              in_=din[f'ca_wk_{i}'].ap().rearrange("(k p) n -> p k n", p=128))
            ssqc_ps = pst(128, TK, "ssqc", tag="o")
            ckraw = []
            for t in range(2):
                k_ps = pst(128, TK, "ckproj")
                for k in range(DCH):
                    nc.tensor.matmul(k_ps[:], lhsT=wkt2[:, k, 128 * t:128 * (t + 1)],
                                     rhs=encb[k][:], start=(k == 0), stop=(k == DCH - 1))
                kr = aqp.tile([128, TK], BF, tag=f"ckr{t}", name=f"ckr{t}")
                nc.scalar.activation(kr[:], k_ps[:], Copy)
                ckraw.append(kr)
                sqc = workA.tile([128, TK], F16, tag="sqc", name="sqc")
                nc.scalar.activation(sqc[:], k_ps[:], Square)
                nc.tensor.matmul(ssqc_ps[:], lhsT=bdall[:, t, :],
                                 rhs=sqc[:], start=(t == 0), stop=(t == 1))
            crr = batch_norm_scale(ssqc_ps, 4, width=TK)
            for t in range(2):
                rbch = pair_bcast(crr, t, nrows=4, width=TK)
                kh = workA.tile([128, TK], FP, tag="khca", name="khca")
                nc.vector.tensor_tensor(kh[:], ckraw[t][:], rbch[:], MUL)
                ckp = workB.tile([128, TK], BF, tag="ckp", name="ckp")
                nc.vector.tensor_scalar(
                    out=ckp[:], in0=kh[:],
                    scalar1=ksc_t[:, t:t + 1], scalar2=None, op0=MUL)
                for half in range(2):
                    kv = 2 * t + half
                    for dd in range(2):
                        nc.scalar.dma_start(out=kdca[kv][64 * dd:64 * (dd + 1), :],
                                            in_=ckp[64 * half:64 * (half + 1), :])

            # -- CA V (4 chunks of 128 enc tokens) --
            wvt2 = wkv.tile([128, DCH, KVD], BF, tag="wkv", name="wkv")
            nc.sync.dma_start(out=wvt2[:],
                              in_=din[f'ca_wv_{i}'].ap().rearrange("(k p) n -> p k n", p=128))
            for kv in range(KVH):
                nc.vector.memset(cv[kv][:, :, 64:65], 1.0)
                nc.vector.memset(cv[kv][:, :, 65:128], 0.0)
            for j in range(TP):
                v_ps = pst(128, KVD, "vproj", tag="m")
                for k in range(DCH):
                    nc.tensor.matmul(v_ps[:], lhsT=encb[k][:, 128 * j:128 * (j + 1)],
                                     rhs=wvt2[:, k, :], start=(k == 0), stop=(k == DCH - 1))
                for kv in range(KVH):
                    nc.scalar.activation(cv[kv][:, j, 0:64],
                                         v_ps[:, 64 * kv:64 * (kv + 1)], Copy)

            if DBG and i == 0:
                nc.sync.dma_start(out=dbg['tap_rbc'].ap(), in_=rbc[:])
                nc.sync.dma_start(out=dbg['tap_xb'].ap(), in_=xb[0][:])
                nc.sync.dma_start(out=dbg['tap_qf'].ap(), in_=qf[0][:])
            sa_attention(i)
            if DBG and i == 0:
                nc.sync.dma_start(out=dbg['tap_ao'].ap(), in_=ao[0][:])
            stream_out_proj(f'sa_wo_{i}')
            if DBG and i == 0:
                nc.sync.dma_start(out=dbg['tap_x1'].ap(), in_=x[0][:])

            # ---- LN2 (deferred) + CA ----
            rbc, rv2 = ln_prep()

            # -- CA Q (8 chunks, no rope) --
            ssqq_ps = pst(128, T, "ssqq", tag="o")
            qrawb = []
            for bki in range(2):
                wt = wbig.tile([128, DCH, 512], BF, tag="wbig", name="wbig")
                nc.sync.dma_start(
                    out=wt[:],
                    in_=din[f'ca_wq_{i}'].ap().rearrange("(k p) n -> p k n", p=128)
                    [:, :, bki * 512:(bki + 1) * 512])
                for jj in range(4):
                    mt = bki * 4 + jj
                    q_ps = pst(128, T, "qproj")
                    for k in range(DCH):
                        nc.tensor.matmul(q_ps[:], lhsT=wt[:, k, 128 * jj:128 * (jj + 1)],
                                         rhs=xb[k][:], start=(k == 0), stop=(k == DCH - 1))
                    qr = aqp.tile([128, T], BF, tag=f"cqr{mt}", name=f"cqr{mt}")
                    nc.scalar.activation(qr[:], q_ps[:], Copy)
                    qrawb.append(qr)
                    sqq = workA.tile([128, T], F16, tag="sqq", name="sqq")
                    nc.scalar.activation(sqq[:], q_ps[:], Square)
                    nc.tensor.matmul(ssqq_ps[:], lhsT=bdall[:, mt, :],
                                     rhs=sqq[:], start=(mt == 0), stop=(mt == 7))
            qsc = batch_norm_scale(ssqq_ps, 16, rv2=rv2)
            for mt in range(DCH):
                rbch = pair_bcast(qsc, mt, rbc=rbc)
                nc.vector.tensor_tensor(qf[mt][:], qrawb[mt][:], rbch[:], MUL)

            ca_attention(i)
            stream_out_proj(f'ca_wo_{i}')

            # ---- LN3 (deferred) + FFN ----
            rbc, rv2 = ln_prep()

            NF = F // 512
            prods = []
            for fb in range(NF):
                wgt = wbig.tile([128, DCH, 512], BF, tag="wbig", name="wbig")
                nc.sync.dma_start(
                    out=wgt[:],
                    in_=din[f'ffn_wg_{i}'].ap().rearrange("(k p) n -> p k n", p=128)
                    [:, :, fb * 512:(fb + 1) * 512])
                wut = wbig.tile([128, DCH, 512], BF, tag="wbig", name="wbig")
                nc.sync.dma_start(
                    out=wut[:],
                    in_=din[f'ffn_wu_{i}'].ap().rearrange("(k p) n -> p k n", p=128)
                    [:, :, fb * 512:(fb + 1) * 512])
                for hf in range(2):
                    gu = []
                    for which, wt in (('g', wgt), ('u', wut)):
                        g_ps = pst(128, 512, "s")
                        for jj in range(2):
                            j = 2 * hf + jj
                            for k in range(DCH):
                                nc.tensor.matmul(
                                    g_ps[:, 256 * jj:256 * (jj + 1)],
                                    lhsT=wt[:, k, 128 * j:128 * (j + 1)],
                                    rhs=xb[k][:], start=(k == 0), stop=(k == DCH - 1))
                        g_sb = ffnp.tile([128, 512], BF, tag=f"relu{which}", name=f"relu{which}")
                        if which == 'g':
                            nc.scalar.activation(g_sb[:], g_ps[:], Relu)
                        else:
                            nc.vector.tensor_scalar(out=g_sb[:], in0=g_ps[:],
                                                    scalar1=0.0, scalar2=None,
                                                    op0=AluOpType.max)
                        gu.append(g_sb)
                    pr = ffnp.tile([128, 512], BF, tag="prod", name="prod", bufs=16)
                    nc.vector.tensor_tensor(pr[:], gu[0][:], gu[1][:], MUL)
                    prods.append(pr)
            # down-proj per output chunk: the full F contraction accumulates in
            # one 32-matmul psum chain (wd column slices stream through the
            # wbig slots freed by the g/u weights), so the per-fb partial-sum
            # adds and their serial layer-tail chain disappear
            NCH = F // 128
            for m in range(DCH):
                wdp = wbig.tile([128, NCH, 128], BF, tag="wbig", name="wdp")
                nc.sync.dma_start(
                    out=wdp[:],
                    in_=din[f'ffn_wd_{i}'].ap().rearrange("(k p) n -> p k n", p=128)
                    [:, :, 128 * m:128 * (m + 1)])
                yp = pst(128, T, "yp", tag="m")
                for ch in range(NCH):
                    pi = 2 * (ch // 4) + (ch % 4) // 2
                    nc.tensor.matmul(
                        yp[:],
                        lhsT=wdp[:, ch, :],
                        rhs=prods[pi][:, 256 * (ch % 2):256 * (ch % 2 + 1)],
                        start=(ch == 0), stop=(ch == NCH - 1))
                nc.vector.tensor_tensor(y_sb[m][:], yp[:], rv2[:], MUL)
                nc.vector.tensor_tensor(x[m][:], x[m][:], y_sb[m][:], ADD)

        # ---- final norm + output ----
        ssf = pst(128, T, "ssf", tag="o")
        for m in range(DCH):
            sq = workA.tile([128, T], F16, tag="sq", name="sq")
            nc.vector.tensor_tensor(sq[:], x[m][:], x[m][:], MUL)
            nc.tensor.matmul(ssf[:], lhsT=ones[:], rhs=sq[:],
                             start=(m == 0), stop=(m == DCH - 1))
        lnf = workB.tile([128, T], FP, tag="lnf", name="lnf")
        nc.scalar.activation(lnf[:], ssf[:], Ln, bias=eps_t[:], scale=1.0 / D)
        rbcf = workA.tile([128, T], FP, tag="rbcf", name="rbcf")
        nc.scalar.activation(rbcf[:], lnf[:], Exp, scale=-0.5)
        for m in range(DCH):
            ot = workB.tile([128, T], FP, tag="otile", name="otile", bufs=2)
            nc.vector.tensor_tensor(ot[:], x[m][:], rbcf[:], MUL)
            nc.vector.tensor_scalar(out=ot[:], in0=ot[:],
                                    scalar1=fscale[:, m:m + 1], scalar2=None, op0=MUL)
            nc.sync.dma_start(out=out_dram.ap()[128 * m:128 * (m + 1), :], in_=ot[:])

    nc.compile()
    return nc


def _get_program():
    global _PROG
    if _PROG is None:
        _PROG = _build_program()
    return _PROG


def kernel(**inputs):
    from concourse import bass_utils
    host, per_core = host_prepare(inputs)
    nc = _get_program()
    in_maps = []
    for c in range(NCORES):
        m = dict(per_core[c])
        for k, v in host.items():
            if k.startswith('rope'):
                continue  # per-core sliced versions already present
            m[k] = v
        in_maps.append(m)
    res = bass_utils.run_bass_kernel_spmd(nc, in_maps, list(range(NCORES)))
    out = np.empty((B, TQ, D), np.float32)
    for c in range(NCORES):
        grp, r = c // TP, c % TP
        out[grp, r::TP] = res.results[c]['outT'].T
    return out

